# revision 38
# baseline (speedup 1.0000x reference)
"""Trainium2 Bass kernel for nn_Attention_54589034332712.

Sharding: 8 cores = 4 batches x 2 head-halves (tensor parallel over heads,
per the sharding hint).  Core c handles batch c//2 and heads
[8*(c%2), 8*(c%2)+8) for all 1024 queries.  Each core computes a partial
output projection over its 8 heads; the halves are summed at gather time
(device collectives fail to load in this environment, so the all-reduce of
the hint happens host-side as part of unsharding).

Mask specialization (exact, derived from the actual mask values at build
time, so any 0/1 mask is handled correctly):
  The reference computes w*mask - finfo.min*(1-mask): masked entries get a
  huge positive bias, so for any query row with >=1 masked entry softmax
  underflows the unmasked weights to exactly 0 and distributes uniformly
  over masked entries.  We compute P_num = exp(scores) + BT where
  BT = C*(1-maskT), C = 2^115.  For q-chunks where ALL rows have >=1 masked
  entry, P_num = BT alone is exact (unmasked weights are exactly 0 in the
  reference), so scores/exp are skipped and the AV matmul consumes BT
  directly.  Blocks with no masked entries skip the BT add.  Denominators
  come free from a ones column appended to V; division uses fp32 reciprocal
  + a rank-1 f32r broadcast matmul.  All matmuls in float32r.
"""

import sys

sys.path.insert(0, "/opt/trn_rl_repo")

import os

import numpy as np
import ml_dtypes

import concourse.bacc as bacc
import concourse.bass as bass
import concourse.mybir as mybir
import concourse.tile as tile
from concourse.bass_utils import run_bass_kernel_spmd

f32 = mybir.dt.float32
f32r = mybir.dt.float32r
bf16 = mybir.dt.bfloat16
u16 = mybir.dt.uint16
u32 = mybir.dt.uint32
Act = mybir.ActivationFunctionType
Alu = mybir.AluOpType

B, S, E, H = 4, 1024, 1024, 16
D = E // H  # 64
HH = H // 2  # heads per core (8)
NG = HH // 2  # local head groups of 2 (4)
EC = E // 128  # contraction chunks (8)
KC = S // 128  # k chunks (8)
TC = S // 128  # position chunks (causal path)
QC = S // 512  # q chunks (2)
MASK_C = float(2.0**115)
N_CORES = 8
ONE_F32_BITS = 1065353216

SC_BUFS = int(os.environ.get("KSC_BUFS", "1"))
EP_BUFS = int(os.environ.get("KEP_BUFS", "6"))
MM_BUFS = int(os.environ.get("KMM_BUFS", "2"))

_program_cache = {}


def classify_mask(attn_mask, bk_zero=True):
    """Per q-chunk execution mode + per-block mask info, uniform across cores.

    Modes per 512-row q-chunk:
      ("degen", None): every row has >=1 masked entry -> P_num = BT exactly
        (reference softmax underflows unmasked weights to exactly 0).
      ("corr", (r0, r1)): like degen except a small contiguous range of rows
        [r0, r1) has no masked entries; those columns get a dense-softmax
        correction accumulated into the AV psum.
      ("full", None): general path (scores+exp for every block, BT add where
        the block has masked entries).
    """
    m = np.asarray(attn_mask) != 0.0  # True = keep
    row_has_masked = ~m.all(axis=1)  # (S,)
    modes = []
    block_has_masked = []
    for qc in range(QC):
        rows = slice(512 * qc, 512 * (qc + 1))
        rhm = row_has_masked[rows]
        live = np.nonzero(~rhm)[0]
        if len(live) == 0:
            modes.append(("degen", None))
        elif bk_zero and len(live) <= 64 and live[-1] - live[0] + 1 == len(live):
            # f32r matmuls need even moving sizes and 8B-aligned starts; pad
            # the range into degenerate rows (their e^S contributions are
            # exactly absorbed by the 2^115 mask terms).
            r0 = int(live[0]) & ~1
            r1 = int(live[-1]) + 1
            w = r1 - r0
            w += w % 2
            if r0 + w > 512:
                r0 = 512 - w
            modes.append(("corr", (r0, r0 + w)))
        else:
            modes.append(("full", None))
        block_has_masked.append(
            tuple(
                bool((~m[rows, 128 * j : 128 * (j + 1)]).any()) for j in range(KC)
            )
        )
    return tuple(modes), tuple(block_has_masked)


def build_program(qc_modes, block_has_masked, bv_zero=False):
    key = (qc_modes, block_has_masked, bv_zero)
    if key in _program_cache:
        return _program_cache[key]
    nc = bacc.Bacc("TRN2", target_bir_lowering=False, debug=False, num_devices=N_CORES)

    hT_d = nc.dram_tensor("hT", [E, S], f32, kind="ExternalInput").ap()
    maskT_d = nc.dram_tensor("maskT", [S, S], mybir.dt.uint8, kind="ExternalInput").ap()
    wqkv_d = nc.dram_tensor("w_qkv_half", [E, 3 * 512], f32, kind="ExternalInput").ap()
    wp_d = nc.dram_tensor("w_proj_half", [512, E], f32, kind="ExternalInput").ap()
    wkT_d = nc.dram_tensor("w_kT_half", [512, E], f32, kind="ExternalInput").ap()
    bqkv_d = nc.dram_tensor("b_qkv_half", [3 * 512], f32, kind="ExternalInput").ap()
    bproj_d = nc.dram_tensor("b_proj_in", [E], f32, kind="ExternalInput").ap()
    out_d = nc.dram_tensor("out", [S, E], f32, kind="ExternalOutput").ap()

    # BT slots needed: for degenerate chunks every j; for live chunks only
    # blocks with masked entries.
    bt_slots = {}
    for qc in range(QC):
        for j in range(KC):
            if qc_modes[qc][0] in ("degen", "corr") or block_has_masked[qc][j]:
                bt_slots[(qc, j)] = len(bt_slots)
    n_bt = max(1, len(bt_slots))

    any_full = any(m == "full" for m, _ in qc_modes)
    any_corr = any(m == "corr" for m, _ in qc_modes)
    ep_bufs = EP_BUFS if any_full else 2
    with tile.TileContext(nc) as tc:
        with (
            tc.tile_pool(name="const", bufs=1) as constp,
            tc.tile_pool(name="qt", bufs=1) as qtp,
            tc.tile_pool(name="kt", bufs=1) as ktp,
            tc.tile_pool(name="vv", bufs=1) as vvp,
            tc.tile_pool(name="bt", bufs=1) as btp,
            tc.tile_pool(name="avall", bufs=1) as avallp,
        ):
            ones_f = constp.tile([1, 128], f32)
            nc.vector.memset(ones_f[:], 1.0)
            ones = constp.tile([1, 128], f32r)
            nc.vector.tensor_copy(ones[:], ones_f[:])
            onescol_f = constp.tile([128, 1], f32)
            nc.vector.memset(onescol_f[:], 1.0)
            ones_col = constp.tile([128, 1], f32r)
            nc.vector.tensor_copy(ones_col[:], onescol_f[:])
            cbias = constp.tile([128, 1], f32)
            nc.vector.memset(cbias[:], MASK_C)

            bqkv_sb = constp.tile([128, 8], f32)  # q,k biases as columns
            nc.sync.dma_start(
                bqkv_sb[:], bqkv_d[0:1024].rearrange("(c p) -> p c", p=128)
            )
            bq_s = constp.tile([128, 4], f32)
            nc.scalar.mul(bq_s[:], bqkv_sb[:, 0:4], 0.125)
            bk_r = constp.tile([128, 4], f32r)
            nc.vector.tensor_copy(bk_r[:], bqkv_sb[:, 4:8])

            bv0 = constp.tile([1, 512], f32r)
            nc.sync.dma_start(
                bv0[:],
                bqkv_d[1024:1536].rearrange("(c t) -> c t", c=1).bitcast(f32r),
            )
            bp0 = constp.tile([1, 512], f32r)
            bp1 = constp.tile([1, 512], f32r)
            nc.sync.dma_start(
                bp0[:], bproj_d[0:512].rearrange("(c t) -> c t", c=1).bitcast(f32r)
            )
            nc.sync.dma_start(
                bp1[:], bproj_d[512:E].rearrange("(c t) -> c t", c=1).bitcast(f32r)
            )

            QT = qtp.tile([128, NG * S], f32r)
            KT = ktp.tile([128, NG * S], f32r)
            V = vvp.tile([128, KC * 512], f32r)  # plain: chunk t, head h at 512t+64h
            BT = btp.tile([128, n_bt * 512], f32r)
            corr_w = {qc: rng[1] - rng[0] for qc, (m, rng) in enumerate(qc_modes) if m == "corr"}
            n_eec = max(1, sum(KC * HH * w for w in corr_w.values()))
            eec_all = btp.tile([128, n_eec], f32r)  # exp'd corr scores, (qc major) j x (h,w)
            av_all = avallp.tile([128, NG * S], f32r)

            wpp_cm = tc.tile_pool(name="wp", bufs=1)
            wpp = wpp_cm.__enter__()
            bpb = wpp.tile([128, E], f32, tag="bpb", name="bproj_bcast")
            wp_t = [
                wpp.tile([128, E], f32r, tag=f"wp{g}", name=f"wp_{g}")
                for g in range(NG)
            ]

            def _emit_wp_dmas():
                for g in range(NG):
                    nc.sync.dma_start(
                        wp_t[g][:], wp_d[128 * g : 128 * (g + 1), :].bitcast(f32r)
                    )

            # --- phase A: load + QKV ---
            with (
                tc.tile_pool(name="ht", bufs=1) as htp,
                tc.tile_pool(name="mstage", bufs=2) as msp,
                tc.tile_pool(name="wqk", bufs=4) as wqkp,
                tc.tile_pool(name="wvp", bufs=1) as wvp,
                tc.tile_pool(name="mm", bufs=MM_BUFS, space="PSUM") as mmps,
            ):
                hT = htp.tile([128, EC * S], f32r)

                def _emit_ht_dmas():
                    for e in range(EC):
                        nc.sync.dma_start(
                            hT[:, S * e : S * (e + 1)],
                            hT_d[128 * e : 128 * (e + 1), :].bitcast(f32r),
                        )
                wv = wvp.tile([128, EC * 512], f32r)

                def _emit_wv_dma():
                    for e in range(EC):
                        nc.sync.dma_start(
                            wv[:, 512 * e : 512 * (e + 1)],
                            wqkv_d[128 * e : 128 * (e + 1), 1024:1536].bitcast(f32r),
                        )

                def _emit_v():
                    for t in range(KC):
                        ps3 = mmps.tile([128, 512], f32, tag="mm")
                        for e in range(EC):
                            nc.tensor.matmul(
                                ps3[:],
                                hT[:, S * e + 128 * t : S * e + 128 * (t + 1)],
                                wv[:, 512 * e : 512 * (e + 1)],
                                start=(e == 0),
                                stop=(bv_zero and e == EC - 1),
                            )
                        if not bv_zero:
                            nc.tensor.matmul(
                                ps3[:], ones[0:1, 0:128], bv0[0:1, :],
                                start=False, stop=True,
                            )
                        nc.vector.tensor_copy(
                            V[:, 512 * t : 512 * (t + 1)], ps3[:]
                        )

                def _emit_qk_dmas(groups):
                    tiles = []
                    for g in groups:
                        wq = wqkp.tile([128, EC * 128], f32r, tag="wq", name=f"wq_{g}")
                        nc.sync.dma_start(
                            wq[:].rearrange("p (c d) -> p c d", d=128),
                            wqkv_d[:, 128 * g : 128 * (g + 1)]
                            .bitcast(f32r)
                            .rearrange("(c p) d -> p c d", p=128),
                        )
                        wk = wqkp.tile([128, EC * 128], f32r, tag="wk", name=f"wk_{g}")
                        if True:
                            nc.sync.dma_start(
                                wk[:].rearrange("p (c d) -> p c d", d=128),
                                wqkv_d[:, 512 + 128 * g : 512 + 128 * (g + 1)]
                                .bitcast(f32r)
                                .rearrange("(c p) d -> p c d", p=128),
                            )
                        tiles.append((wq, wk))
                    return tiles

                def _emit_wkT_dmas():
                    tiles = []
                    for g in range(NG):
                        wkt = wqkp.tile([128, E], f32r, tag="wkt", name=f"wkt_{g}")
                        nc.sync.dma_start(
                            wkt[:], wkT_d[128 * g : 128 * (g + 1), :].bitcast(f32r)
                        )
                        tiles.append(wkt)
                    return tiles


                def _emit_qk():
                    for g in range(NG):
                        wq, wk = _qk_tiles[g]
                        for t in range(QC):
                            mode_t, rng_t = qc_modes[t]
                            if mode_t == "full":
                                ps = mmps.tile([128, 512], f32, tag="mm")
                                for e in range(EC):
                                    nc.tensor.matmul(
                                        ps[:],
                                        wq[:, 128 * e : 128 * (e + 1)],
                                        hT[:, S * e + 512 * t : S * e + 512 * (t + 1)],
                                        start=(e == 0),
                                        stop=(e == EC - 1),
                                    )
                                nc.scalar.activation(
                                    QT[:, S * g + 512 * t : S * g + 512 * (t + 1)],
                                    ps[:],
                                    Act.Identity,
                                    bias=bq_s[:, g : g + 1],
                                    scale=0.125,
                                )
                            elif mode_t == "corr":
                                # only the live correction columns are consumed
                                r0, r1 = rng_t
                                w = r1 - r0
                                psl = mmps.tile(
                                    [128, w], f32, tag="mml", name=f"psl_{g}_{t}"
                                )
                                for e in range(EC):
                                    nc.tensor.matmul(
                                        psl[:],
                                        wq[:, 128 * e : 128 * (e + 1)],
                                        hT[
                                            :,
                                            S * e + 512 * t + r0 : S * e + 512 * t + r1,
                                        ],
                                        start=(e == 0),
                                        stop=(e == EC - 1),
                                    )
                                nc.scalar.activation(
                                    QT[
                                        :,
                                        S * g + 512 * t + r0 : S * g + 512 * t + r1,
                                    ],
                                    psl[:],
                                    Act.Identity,
                                    bias=bq_s[:, g : g + 1],
                                    scale=0.125,
                                )
                            if True:
                                ps2 = mmps.tile([128, 512], f32, tag="mm")
                                for e in range(EC):
                                    nc.tensor.matmul(
                                        ps2[:],
                                        wk[:, 128 * e : 128 * (e + 1)],
                                        hT[:, S * e + 512 * t : S * e + 512 * (t + 1)],
                                        start=(e == 0),
                                        stop=(e == EC - 1),
                                    )
                                nc.scalar.activation(
                                    KT[:, S * g + 512 * t : S * g + 512 * (t + 1)],
                                    ps2[:],
                                    Act.Identity,
                                    bias=bqkv_sb[:, 4 + g : 5 + g],
                                    scale=1.0,
                                )

                # priority order: hT (everything), wv + mask (the AV wave
                # needs only V and BT), then the QK weights (corrections only)
                def _emit_corr_scores(wkt_tiles):
                    ofs = 0
                    for qc in range(QC):
                        mode_t, rng_t = qc_modes[qc]
                        if mode_t != "corr":
                            continue
                        r0, r1 = rng_t
                        w = r1 - r0
                        hw = HH * w
                        for g in range(NG):
                            for s in range(2):
                                hloc = 2 * g + s
                                scc = mmps.tile(
                                    [128, KC * w], f32, tag="ups", name=f"scc_{qc}_{g}_{s}"
                                )
                                for j in range(KC):
                                    nc.tensor.matmul(
                                        scc[:, j * w : (j + 1) * w],
                                        KT[
                                            64 * s : 64 * (s + 1),
                                            S * g + 128 * j : S * g + 128 * (j + 1),
                                        ],
                                        QT[
                                            64 * s : 64 * (s + 1),
                                            S * g + 512 * qc + r0 : S * g + 512 * qc + r1,
                                        ],
                                        start=True,
                                        stop=True,
                                        skip_group_check=True,
                                    )
                                eout = (
                                    eec_all[:, ofs : ofs + KC * hw]
                                    .rearrange("p (j hh) -> p j hh", hh=hw)
                                    [:, :, w * hloc : w * (hloc + 1)]
                                )
                                nc.scalar.activation(
                                    eout,
                                    scc[:].rearrange("p (j wi) -> p j wi", wi=w),
                                    Act.Exp,
                                )
                        ofs += KC * hw

                def _emit_mask():
                    for (qc, j), slot in bt_slots.items():
                        mst = msp.tile([128, 512], mybir.dt.uint8, tag="mst", name=f"mst_{qc}_{j}")
                        nc.sync.dma_start(
                            mst[:],
                            maskT_d[128 * j : 128 * (j + 1), 512 * qc : 512 * (qc + 1)],
                        )
                        nc.scalar.activation(
                            BT[:, 512 * slot : 512 * (slot + 1)],
                            mst[:],
                            Act.Identity,
                            bias=cbias[:],
                            scale=-MASK_C,
                        )

                if any_full:
                    _qk_tiles = _emit_qk_dmas([0])
                    _emit_ht_dmas()
                    _qk_tiles += _emit_qk_dmas([1, 2, 3])
                    _emit_wv_dma()
                    _emit_mask()
                    _emit_qk()
                    _emit_v()
                    if any_corr:
                        _emit_corr_scores(None)
                    _emit_wp_dmas()
                else:
                    _emit_ht_dmas()
                    _emit_wv_dma()
                    _emit_mask()
                    _emit_v()
                    _qk_tiles = _emit_qk_dmas([0, 1, 2, 3])
                    _emit_qk()
                    if any_corr:
                        _emit_corr_scores(None)
                    _emit_wp_dmas()
            # --- phase B: attention (+ projection, same scope for overlap) ---
            with (
                tc.tile_pool(name="outp", bufs=4) as outp,
                tc.tile_pool(name="mm2", bufs=2, space="PSUM") as mmps2,
                tc.tile_pool(name="sc", bufs=SC_BUFS, space="PSUM") as scps,
                tc.tile_pool(name="avps", bufs=int(os.environ.get("KAV_BUFS","2")), space="PSUM") as avps,
                tc.tile_pool(name="bc", bufs=1, space="PSUM") as bcps,
                tc.tile_pool(name="ee", bufs=ep_bufs) as eep,
                tc.tile_pool(name="pp", bufs=ep_bufs) as ppp,
                tc.tile_pool(name="avtmp", bufs=2) as avtp,
                tc.tile_pool(name="rc", bufs=1) as rcp,
            ):
                recips = rcp.tile([1, HH * QC * 512], f32r)
                btden_sb = rcp.tile([1, QC * 512], f32r)
                btdraw_sb = rcp.tile([1, QC * 512], f32)
                bcast_sb = rcp.tile([128, QC * 512], f32)
                for c in range(2):
                    bq_ps = bcps.tile([128, 512], f32, tag="bc", name=f"bpb_{c}")
                    nc.tensor.matmul(
                        bq_ps[:],
                        ones[0:1, 0:128],
                        (bp0 if c == 0 else bp1)[0:1, :],
                        start=True,
                        stop=True,
                    )
                    nc.scalar.copy(bpb[:, 512 * c : 512 * (c + 1)], bq_ps[:])
                # shared denominators for BT-direct chunks: Sum_k BT[k, q]
                for qc in range(QC):
                    mode, rng = qc_modes[qc]
                    if mode == "full":
                        continue
                    btd = bcps.tile([1, 512], f32, tag="bc", name=f"btd_{qc}")
                    for j in range(KC):
                        nc.tensor.matmul(
                            btd[:],
                            ones_col[:],
                            BT[:, 512 * bt_slots[(qc, j)] : 512 * (bt_slots[(qc, j)] + 1)],
                            start=(j == 0),
                            stop=(j == KC - 1),
                        )
                    nc.scalar.copy(btdraw_sb[0:1, 512 * qc : 512 * (qc + 1)], btd[:])
                    with nc.allow_low_precision(reason="f32r recip for bcast"):
                        nc.vector.reciprocal(
                            btden_sb[0:1, 512 * qc : 512 * (qc + 1)], btd[:]
                        )
                    bcq = bcps.tile([128, 512], f32, tag="bc", name=f"bcq_{qc}")
                    nc.tensor.matmul(
                        bcq[:],
                        ones[0:1, 0:128],
                        btden_sb[0:1, 512 * qc : 512 * (qc + 1)],
                        start=True,
                        stop=True,
                    )
                    nc.scalar.copy(bcast_sb[:, 512 * qc : 512 * (qc + 1)], bcq[:])

                for g in range(NG):
                    for qc in range(QC):
                        mode, rng = qc_modes[qc]
                        if mode in ("degen", "corr"):
                            av2 = avps.tile([128, 512], f32, tag="av", name=f"av2_{g}_{qc}")
                            for j in range(KC):
                                nc.tensor.matmul(
                                    av2[:],
                                    V[:, 512 * j + 128 * g : 512 * j + 128 * (g + 1)],
                                    BT[
                                        :,
                                        512 * bt_slots[(qc, j)] : 512
                                        * (bt_slots[(qc, j)] + 1),
                                    ],
                                    start=(j == 0),
                                    stop=(j == KC - 1),
                                )
                            cds_s = []
                            cps_s = []
                            if mode == "corr":
                                r0, r1 = rng
                                w = r1 - r0
                                hw = HH * w
                                ofs = 0
                                for q2 in range(qc):
                                    if qc_modes[q2][0] == "corr":
                                        ofs += KC * HH * (qc_modes[q2][1][1] - qc_modes[q2][1][0])
                                for s in range(2):
                                    hloc = 2 * g + s
                                    cps = avps.tile(
                                        [64, w], f32, tag="cps", name=f"cps{g}{qc}{s}"
                                    )
                                    for j in range(KC):
                                        nc.tensor.matmul(
                                            cps[:],
                                            V[
                                                :,
                                                512 * j + 128 * g + 64 * s : 512 * j
                                                + 128 * g
                                                + 64 * (s + 1),
                                            ],
                                            eec_all[
                                                :,
                                                ofs + hw * j + w * hloc : ofs
                                                + hw * j
                                                + w * (hloc + 1),
                                            ],
                                            start=(j == 0),
                                            stop=(j == KC - 1),
                                        )
                                    cps_s.append(cps)
                                    # per-head denominator delta for corr cols
                                    cd = bcps.tile(
                                        [1, w], f32, tag="bc", name=f"cd{g}{qc}{s}"
                                    )
                                    for j in range(KC):
                                        nc.tensor.matmul(
                                            cd[:],
                                            ones_col[:],
                                            eec_all[
                                                :,
                                                ofs + hw * j + w * hloc : ofs
                                                + hw * j
                                                + w * (hloc + 1),
                                            ],
                                            start=(j == 0),
                                            stop=(j == KC - 1),
                                            skip_group_check=True,
                                        )
                                    cds = avtp.tile(
                                        [1, w], f32, tag=f"cds{s}", name=f"cds{g}{qc}{s}"
                                    )
                                    nc.vector.tensor_copy(cds[:], cd[:])
                                    cds_s.append(cds)
                            # divide (writes garbage into corr cols; fixed below)
                            nc.vector.tensor_tensor(
                                out=av_all[:, S * g + 512 * qc : S * g + 512 * (qc + 1)],
                                in0=av2[:],
                                in1=bcast_sb[:, 512 * qc : 512 * (qc + 1)],
                                op=Alu.mult,
                            )
                            if mode == "corr":
                                r0, r1 = rng
                                w = r1 - r0
                                for s in range(2):
                                    # combined numerator: BT part (av2) + e^S
                                    # part (cps); fp32 add swallows exactly the
                                    # right one on both degenerate-padded and
                                    # live rows.
                                    cps_sb = avtp.tile(
                                        [64, w], f32, tag="cpssb", name=f"cb{g}{qc}{s}"
                                    )
                                    nc.scalar.copy(cps_sb[:], cps_s[s][:])
                                    val_sb = avtp.tile(
                                        [64, w], f32, tag="valsb", name=f"vl{g}{qc}{s}"
                                    )
                                    nc.vector.tensor_tensor(
                                        out=val_sb[:],
                                        in0=av2[64 * s : 64 * (s + 1), r0:r1],
                                        in1=cps_sb[:],
                                        op=Alu.add,
                                    )
                                    dcomb = avtp.tile(
                                        [1, w], f32, tag="dcomb", name=f"dc{g}{qc}{s}"
                                    )
                                    nc.vector.tensor_tensor(
                                        out=dcomb[:],
                                        in0=cds_s[s][:],
                                        in1=btdraw_sb[
                                            0:1,
                                            512 * qc + r0 : 512 * qc + r1,
                                        ],
                                        op=Alu.add,
                                    )
                                    rcw = avtp.tile(
                                        [1, w], f32r, tag="rcw", name=f"rcw{g}{qc}{s}"
                                    )
                                    with nc.allow_low_precision(reason="recip"):
                                        nc.vector.reciprocal(rcw[:], dcomb[:])
                                    bcw = bcps.tile(
                                        [64, w], f32, tag="bc", name=f"bcw{g}{qc}{s}"
                                    )
                                    nc.tensor.matmul(
                                        bcw[:], ones[0:1, 0:64], rcw[:],
                                        start=True, stop=True,
                                    )
                                    bcw_sb = avtp.tile(
                                        [64, w], f32, tag="bcwsb", name=f"bw{g}{qc}{s}"
                                    )
                                    nc.scalar.copy(bcw_sb[:], bcw[:])
                                    nc.vector.tensor_tensor(
                                        out=av_all[
                                            64 * s : 64 * (s + 1),
                                            S * g + 512 * qc + r0 : S * g
                                            + 512 * qc
                                            + r1,
                                        ],
                                        in0=val_sb[:],
                                        in1=bcw_sb[:],
                                        op=Alu.mult,
                                    )
                        else:
                            # full path: per-head scores/exp/(BT add)/AV + denom
                            # (s passes deinterleaved so one dn slot suffices)
                            av_t = [
                                avps.tile([64, 512], f32, tag="av", name=f"avf_{g}_{qc}_{s}")
                                for s in range(2)
                            ]
                            for s in range(2):
                                dn = bcps.tile(
                                    [1, 512], f32, tag="bc", name=f"dn_{g}_{qc}_{s}"
                                )
                                for j in range(KC):
                                    sc = scps.tile(
                                        [128, 512], f32, tag="sc", name=f"sc{g}{qc}{j}{s}"
                                    )
                                    nc.tensor.matmul(
                                        sc[:],
                                        KT[
                                            64 * s : 64 * (s + 1),
                                            S * g + 128 * j : S * g + 128 * (j + 1),
                                        ],
                                        QT[
                                            64 * s : 64 * (s + 1),
                                            S * g + 512 * qc : S * g + 512 * (qc + 1),
                                        ],
                                        start=True,
                                        stop=True,
                                    )
                                    ee = eep.tile(
                                        [128, 512], f32r, tag="ee", name=f"ee{g}{qc}{j}{s}"
                                    )
                                    nc.scalar.activation(ee[:], sc[:], Act.Exp)
                                    if block_has_masked[qc][j]:
                                        pp = ppp.tile(
                                            [128, 512], f32r, tag="pp",
                                            name=f"pp{g}{qc}{j}{s}",
                                        )
                                        nc.vector.tensor_tensor(
                                            out=pp[:],
                                            in0=ee[:],
                                            in1=BT[
                                                :,
                                                512 * bt_slots[(qc, j)] : 512
                                                * (bt_slots[(qc, j)] + 1),
                                            ],
                                            op=Alu.add,
                                        )
                                        rhs = pp[:]
                                    else:
                                        rhs = ee[:]
                                    nc.tensor.matmul(
                                        av_t[s][:],
                                        V[
                                            :,
                                            512 * j + 128 * g + 64 * s : 512 * j
                                            + 128 * g
                                            + 64 * (s + 1),
                                        ],
                                        rhs,
                                        start=(j == 0),
                                        stop=(j == KC - 1),
                                    )
                                    nc.tensor.matmul(
                                        dn[:],
                                        ones_col[:],
                                        rhs,
                                        start=(j == 0),
                                        stop=(j == KC - 1),
                                        skip_group_check=True,
                                    )
                                hq = (2 * g + s) * QC + qc
                                with nc.allow_low_precision(reason="recip"):
                                    nc.vector.reciprocal(
                                        recips[0:1, 512 * hq : 512 * (hq + 1)],
                                        dn[:],
                                    )
                            for s in range(2):
                                hq = (2 * g + s) * QC + qc
                                bc = bcps.tile(
                                    [64, 512], f32, tag="bc", name=f"bcf{g}{qc}{s}"
                                )
                                nc.tensor.matmul(
                                    bc[:],
                                    ones[0:1, 0:64],
                                    recips[0:1, 512 * hq : 512 * (hq + 1)],
                                    start=True,
                                    stop=True,
                                )
                                avt = avtp.tile(
                                    [64, 512], f32, tag="avt", name=f"avtf{g}{qc}{s}"
                                )
                                nc.vector.tensor_copy(avt[:], av_t[s][:])
                                nc.vector.tensor_tensor(
                                    out=av_all[
                                        64 * s : 64 * (s + 1),
                                        S * g + 512 * qc : S * g + 512 * (qc + 1),
                                    ],
                                    in0=avt[:],
                                    in1=bc[:],
                                    op=Alu.mult,
                                )

                for t in range(KC):
                    out_sb = outp.tile([128, E], f32, tag="out", name=f"out_{t}")
                    for c in range(2):
                        if (2 * t + c) % 2 == 0:
                            ps = mmps2.tile([128, 512], f32, tag="mm2", name=f"pj_{t}_{c}")
                        else:
                            ps = avps.tile([128, 512], f32, tag="cps", name=f"pj_{t}_{c}")
                        for g in range(NG):
                            nc.tensor.matmul(
                                ps[:],
                                av_all[:, S * g + 128 * t : S * g + 128 * (t + 1)],
                                wp_t[g][:, 512 * c : 512 * (c + 1)],
                                start=(g == 0),
                                stop=(g == NG - 1),
                            )
                        nc.vector.scalar_tensor_tensor(
                            out=out_sb[:, 512 * c : 512 * (c + 1)],
                            in0=ps[:],
                            scalar=1.0,
                            in1=bpb[:, 512 * c : 512 * (c + 1)],
                            op0=Alu.mult,
                            op1=Alu.add,
                        )
                        nc.sync.dma_start(
                            out_d[128 * t : 128 * (t + 1), 512 * c : 512 * (c + 1)],
                            out_sb[:, 512 * c : 512 * (c + 1)],
                        )
            wpp_cm.__exit__(None, None, None)

    nc.compile()
    _program_cache[key] = nc
    return nc


N_WARM = int(os.environ.get("KN_WARM", "7"))

# const blob layouts
CB_TRI = 0            # [128, 128] strict-upper (k>q) valued C, bf16
CB_ESEL = 128         # [128, 64]: per t, col block [8] with col t' = (t'==t)
CB_TSEL = 192         # [8, 1024]: per t, [8,128] block, row t' = C*(t'>t)
CB_ONEC = 1216        # [128, 1] ones column
CB_ID = 1220          # [128, 128] identity (for PE transpose)
CB_W = 1348

CF_RECIPN = 0         # [1, 1024] 1/(C*(S-1-q)), 0 at q=S-1
CF_DENC = 1024        # [1, 16] per (g,s,q) denominator constant: C for q=0, 0 for q=1
CF_ONES = 1040        # [1, 128] ones (f32)
CF_BV = 1168          # [1, 512] b_v half
CF_BP = 1680          # [1, 1024] b_proj (zeroed on half 1)
CF_W = 2704


def host_const_blobs(b_qkv, b_proj, hh):
    """Per-core const blobs (bf16 blob as uint16 view, f32 row blob)."""
    cb = np.zeros((128, CB_W), dtype=ml_dtypes.bfloat16)
    k_idx = np.arange(128)[:, None]
    q_idx = np.arange(128)[None, :]
    cb[:, CB_TRI:CB_TRI + 128] = np.where(k_idx > q_idx, MASK_C, 0.0)
    for t in range(TC):
        cb[:, CB_ESEL + 8 * t + t] = 1.0
        tp = np.arange(8)[:, None]
        cb[0:8, CB_TSEL + 128 * t:CB_TSEL + 128 * (t + 1)] = np.where(
            tp > t, MASK_C, 0.0
        )
    cb[:, CB_ONEC] = 1.0
    cb[:, CB_ID:CB_ID + 128] = np.eye(128, dtype=np.float32)

    cf = np.zeros((1, CF_W), dtype=np.float32)
    n = (S - 1) - np.arange(S).astype(np.float64)
    with np.errstate(divide="ignore"):
        recipn = np.where(n > 0, 1.0 / (MASK_C * np.maximum(n, 1.0)), 0.0)
    cf[0, CF_RECIPN:CF_RECIPN + S] = recipn.astype(np.float32)
    denc = np.zeros(16, dtype=np.float32)
    denc[0::2] = MASK_C  # q==0 columns (row S-2): one masked key
    cf[0, CF_DENC:CF_DENC + 16] = denc
    cf[0, CF_ONES:CF_ONES + 128] = 1.0
    cf[0, CF_BV:CF_BV + 512] = b_qkv[2 * E + 512 * hh:2 * E + 512 * (hh + 1)]
    if hh == 0:
        cf[0, CF_BP:CF_BP + E] = b_proj
    bq = b_qkv[512 * hh:512 * (hh + 1)].astype(np.float32)
    bqs = np.ascontiguousarray(0.125 * bq.reshape(4, 128).T)
    rcol = np.where(n > 0, 1.0 / (MASK_C * np.maximum(n, 1.0)), 1.0)
    rcol[S - 2] = 1.0  # last two queries are pre-divided by the fixup
    rcol = np.ascontiguousarray(rcol.astype(np.float32).reshape(TC, 128).T)
    return (np.ascontiguousarray(cb.view(np.uint16)), np.ascontiguousarray(cf),
            bqs, rcol)


def build_program_causal(bq_zero=True, bv_zero=True, bp_zero=True):
    nc = bacc.Bacc("TRN2", target_bir_lowering=False, debug=False,
                   num_devices=N_CORES)

    hT_d = nc.dram_tensor("hT16", [E, S], u16, kind="ExternalInput").ap()
    wv_d = nc.dram_tensor("wv16", [E, 512], u16, kind="ExternalInput").ap()
    wq_d = nc.dram_tensor("wq16", [E, 512], u16, kind="ExternalInput").ap()
    wkT_d = nc.dram_tensor("wkT16", [512, E], u16, kind="ExternalInput").ap()
    wp_d = nc.dram_tensor("wp16", [512, E], u16, kind="ExternalInput").ap()
    cb_d = nc.dram_tensor("cb16", [128, CB_W], u16, kind="ExternalInput").ap()
    cf_d = nc.dram_tensor("cf32", [1, CF_W], f32, kind="ExternalInput").ap()
    rcol_d = nc.dram_tensor("rcol32", [128, TC], f32, kind="ExternalInput").ap()
    if not bq_zero:
        bqs_d = nc.dram_tensor("bqs32", [128, 4], f32, kind="ExternalInput").ap()
    out_d = nc.dram_tensor("out", [S, E], u16, kind="ExternalOutput").ap()

    need_ones = (not bv_zero) or (not bp_zero)

    with tile.TileContext(nc) as tc:
        with (
            tc.tile_pool(name="const", bufs=1) as constp,
            tc.tile_pool(name="big", bufs=1) as bigp,
            tc.tile_pool(name="outp", bufs=4) as outp,
        ):
            # ---- SBUF tiles ----
            cbt = constp.tile([128, CB_W], bf16)
            cft = constp.tile([1, CF_W], f32)
            rcol = constp.tile([128, TC], f32)
            if not bq_zero:
                bqs = constp.tile([128, 4], f32)
            ones_f = constp.tile([128, 128], f32)  # warmup operand (memset)
            qbd = constp.tile([128, 16], bf16)
            Usb = constp.tile([128, EC * 16], bf16)
            eec = constp.tile([128, TC * 16], bf16)
            Bsb = constp.tile([8, 512], bf16)
            rr_sb = constp.tile([1, 16], f32)
            dcomb_sb = constp.tile([1, 16], f32)
            bcw_all = constp.tile([64, 16], f32)
            psl_sb = constp.tile([128, 8], bf16)
            BTsb = constp.tile([128, 4 * TC], bf16)
            R = constp.tile([128, EC * TC], f32)
            Rb = constp.tile([128, EC * TC], bf16)
            if need_ones:
                onesr = constp.tile([1, 128], f32r)
            if not bv_zero:
                bv_r = constp.tile([1, 512], f32r)
            if not bp_zero:
                bp_r = constp.tile([1, E], f32r)
                bpb = constp.tile([128, E], f32)

            hT = bigp.tile([128, EC * S], bf16)
            wvt = bigp.tile([128, EC * 512], bf16)
            wqt = bigp.tile([128, EC * 512], bf16)
            wkTt = bigp.tile([128, NG * E], bf16)
            wpt = bigp.tile([128, NG * E], bf16)
            V = bigp.tile([128, TC * 512], bf16)
            av_all = bigp.tile([128, NG * S], bf16)

            TRI = cbt[:, CB_TRI:CB_TRI + 128]
            onec_bf = cbt[:, CB_ONEC:CB_ONEC + 1]

            # ---- DMA issue (SP: hT; Act: wv + consts + weights) ----
            nc.gpsimd.memset(ones_f[:], 1.0)  # warmup operand, Pool, no DMA dep
            nc.gpsimd.memset(qbd[:], 0.0)
            for e in range(EC):
                nc.scalar.dma_start(
                    wvt[:, 512 * e:512 * (e + 1)],
                    wv_d[128 * e:128 * (e + 1), :].bitcast(bf16),
                )
                nc.sync.dma_start(
                    hT[:, S * e:S * (e + 1)],
                    hT_d[128 * e:128 * (e + 1), :].bitcast(bf16),
                )
            nc.scalar.dma_start(cft[:], cf_d)
            nc.scalar.dma_start(rcol[:], rcol_d)
            if not bq_zero:
                nc.scalar.dma_start(bqs[:], bqs_d)
            nc.scalar.dma_start(
                wqt[:].rearrange("p (c d) -> p c d", d=512),
                wq_d.bitcast(bf16).rearrange("(c p) d -> p c d", p=128),
            )
            nc.scalar.dma_start(cbt[:], cb_d.bitcast(bf16))
            for c in range(2):
                nc.scalar.dma_start(
                    wpt[:].rearrange("p (g e) -> p g e", e=E)[:, :, 512 * c:512 * (c + 1)],
                    wp_d.bitcast(bf16).rearrange("(g p) e -> p g e", p=128)[
                        :, :, 512 * c:512 * (c + 1)
                    ],
                )
            nc.scalar.dma_start(
                wkTt[:].rearrange("p (g e) -> p g e", e=E),
                wkT_d.bitcast(bf16).rearrange("(g p) e -> p g e", p=128),
            )
            for e in range(EC):
                nc.vector.tensor_reduce(
                    R[:, TC * e:TC * (e + 1)],
                    hT[:, S * e:S * (e + 1)].rearrange("p (t k) -> p t k", k=128),
                    axis=mybir.AxisListType.X,
                    op=Alu.add,
                )
            nc.vector.tensor_copy(Rb[:], R[:])
            if need_ones:
                nc.vector.tensor_copy(onesr[:], cft[0:1, CF_ONES:CF_ONES + 128])
            if not bv_zero:
                nc.vector.tensor_copy(bv_r[:], cft[0:1, CF_BV:CF_BV + 512])
            if not bp_zero:
                nc.vector.tensor_copy(bp_r[:], cft[0:1, CF_BP:CF_BP + E])

            # ---- phase A: warmup + V projection (8 psum banks) ----
            with tc.tile_pool(name="vps", bufs=8, space="PSUM") as vpool:
                warm = vpool.tile([128, 512], f32, tag="vp", name="warm")
                for i in range(N_WARM):
                    nc.tensor.matmul(
                        warm[:, 0:128], ones_f[:], ones_f[:],
                        start=True, stop=True,
                    )
                vt = [
                    vpool.tile([128, 512], f32, tag="vp", name=f"v_{t}")
                    for t in range(TC)
                ]
                for e in range(EC - 2):
                    for t in range(TC):
                        nc.tensor.matmul(
                            vt[t][:],
                            hT[:, S * e + 128 * t:S * e + 128 * (t + 1)],
                            wvt[:, 512 * e:512 * (e + 1)],
                            start=(e == 0),
                            stop=False,
                        )
                # tail skew: finish each tile and launch its copy early
                for t in range(TC):
                    for e in (EC - 2, EC - 1):
                        nc.tensor.matmul(
                            vt[t][:],
                            hT[:, S * e + 128 * t:S * e + 128 * (t + 1)],
                            wvt[:, 512 * e:512 * (e + 1)],
                            start=False,
                            stop=(bv_zero and e == EC - 1),
                        )
                    if not bv_zero:
                        nc.tensor.matmul(
                            vt[t][:], onesr[:], bv_r[:], start=False, stop=True,
                        )
                    dst = V[:, 512 * t:512 * (t + 1)]
                    if t % 2 == 0:
                        nc.scalar.copy(dst, vt[t][:])
                    else:
                        nc.vector.tensor_copy(dst, vt[t][:])

                # block column sums, transposed: B^T[d, t'] = Wv^T R
                # (tiny free dim), then PE-transpose back to [t', d]
                pslt = vpool.tile([128, 512], f32, tag="vp", name="pslt")
                for gd in range(4):
                    for e in range(EC):
                        nc.tensor.matmul(
                            pslt[:, 16 + 8 * gd:24 + 8 * gd],
                            wvt[:, 512 * e + 128 * gd:512 * e + 128 * (gd + 1)],
                            Rb[:, TC * e:TC * (e + 1)],
                            start=(e == 0),
                            stop=(e == EC - 1),
                            skip_group_check=True,
                        )
                nc.scalar.copy(BTsb[:], pslt[:, 16:48])
                bsbT = vpool.tile([8, 512], bf16, tag="vp", name="bsbT")
                for gd in range(4):
                    nc.tensor.transpose(
                        bsbT[:, 128 * gd:128 * (gd + 1)],
                        BTsb[:, 8 * gd:8 * (gd + 1)],
                        cbt[:, CB_ID:CB_ID + 128],
                    )
                nc.scalar.copy(Bsb[:, 0:256], bsbT[:, 0:256])
                nc.vector.tensor_copy(Bsb[:, 256:512], bsbT[:, 256:512])
                for g in range(NG):
                    for e in range(EC):
                        nc.tensor.matmul(
                            pslt[:, 2 * g:2 * g + 2],
                            wqt[:, 512 * e + 128 * g:512 * e + 128 * (g + 1)],
                            hT[:, S * e + (S - 2):S * e + S],
                            start=(e == 0),
                            stop=(e == EC - 1),
                            skip_group_check=True,
                        )
                if bq_zero:
                    nc.scalar.activation(psl_sb[:], pslt[:, 0:8], Act.Identity, scale=0.125)
                else:
                    for g in range(NG):
                        nc.scalar.activation(
                            psl_sb[:, 2 * g:2 * g + 2], pslt[:, 2 * g:2 * g + 2],
                            Act.Identity, scale=0.125,
                            bias=bqs[:, g:g + 1],
                        )
                for g in range(NG):
                    for s in range(2):
                        nc.gpsimd.tensor_copy(
                            qbd[64 * s:64 * (s + 1), 4 * g + 2 * s:4 * g + 2 * s + 2],
                            psl_sb[64 * s:64 * (s + 1), 2 * g:2 * g + 2],
                        )

            # ---- phase B ----
            with (
                tc.tile_pool(name="mm", bufs=3, space="PSUM") as mmp,
                tc.tile_pool(name="avp", bufs=4, space="PSUM") as avp,
                tc.tile_pool(name="sm", bufs=1, space="PSUM") as smp,
            ):
                # one psum bank shared by all small tiles, via column regions
                sm = smp.tile([128, 512], f32, tag="sm")
                sm_psl = lambda g: sm[:, 2 * g:2 * g + 2]
                sm_u = lambda e: sm[:, 32 + 16 * e:48 + 16 * e]
                sm_s = lambda j: sm[:, 160 + 16 * j:176 + 16 * j]
                sm_cd = sm[0:1, 288:304]
                sm_bcw = lambda g, s: sm[0:64, 304 + 2 * (2 * g + s):306 + 2 * (2 * g + s)]

                if not bp_zero:
                    for c in range(2):
                        bps = mmp.tile([128, 512], f32, tag="mm", name=f"bpb_{c}")
                        nc.tensor.matmul(
                            bps[:], onesr[:], bp_r[0:1, 512 * c:512 * (c + 1)],
                            start=True, stop=True,
                        )
                        nc.scalar.copy(bpb[:, 512 * c:512 * (c + 1)], bps[:])

                # qc0 attention numerators (queries 0..511), C-scaled
                # NOTE: keep each region's start..stop pair adjacent: interleaving
                # open accumulation groups across tiles miscomputes in this stack.
                for g in range(NG):
                    avt = avp.tile([128, 512], f32, tag="av", name=f"av0_{g}")
                    for tl in range(4):
                        nc.tensor.matmul(
                            avt[:, 128 * tl:128 * (tl + 1)],
                            V[:, 512 * tl + 128 * g:512 * tl + 128 * (g + 1)],
                            TRI,
                            start=True, stop=False,
                            skip_group_check=True,
                        )
                        nc.tensor.matmul(
                            avt[:, 128 * tl:128 * (tl + 1)],
                            Bsb[:, 128 * g:128 * (g + 1)],
                            cbt[0:8, CB_TSEL + 128 * tl:CB_TSEL + 128 * (tl + 1)],
                            start=False, stop=True,
                            skip_group_check=True,
                        )
                    nc.scalar.copy(
                        av_all[:, S * g:S * g + 256], avt[:, 0:256])
                    nc.vector.tensor_copy(
                        av_all[:, S * g + 256:S * g + 512], avt[:, 256:512])

                # u = Wk^T q (block-diagonal via zero-padded qbd)
                for e in range(EC):
                    ups = sm_u(e)
                    for g in range(NG):
                        nc.tensor.matmul(
                            ups[:, 4 * g:4 * (g + 1)],
                            wkTt[:, E * g + 128 * e:E * g + 128 * (e + 1)],
                            qbd[:, 4 * g:4 * (g + 1)],
                            start=True, stop=True,
                            skip_group_check=True,
                        )
                nc.scalar.copy(Usb[:], sm[:, 32:160])

                # s = H^T u; exp -> eec
                for j in range(TC):
                    sps = sm_s(j)
                    for e in range(EC):
                        nc.tensor.matmul(
                            sps,
                            hT[:, S * e + 128 * j:S * e + 128 * (j + 1)],
                            Usb[:, 16 * e:16 * (e + 1)],
                            start=(e == 0),
                            stop=(e == EC - 1),
                            skip_group_check=True,
                        )
                nc.scalar.activation(eec[:], sm[:, 160:288], Act.Exp)

                # denominators for the live row (+ absorbed degenerate col)
                cd = sm_cd
                for j in range(TC):
                    nc.tensor.matmul(
                        cd, onec_bf, eec[:, 16 * j:16 * (j + 1)],
                        start=(j == 0), stop=(j == TC - 1),
                        skip_group_check=True,
                    )
                nc.vector.tensor_tensor(
                    out=dcomb_sb[:], in0=cd,
                    in1=cft[0:1, CF_DENC:CF_DENC + 16], op=Alu.add,
                )
                with nc.allow_low_precision(reason="corr recip"):
                    nc.vector.reciprocal(rr_sb[:], dcomb_sb[:])

                # qc1 attention numerators (queries 512..1023) + correction
                for g in range(NG):
                    avt = avp.tile([128, 512], f32, tag="av", name=f"av1_{g}")
                    for tl in range(4):
                        t = 4 + tl
                        nc.tensor.matmul(
                            avt[:, 128 * tl:128 * (tl + 1)],
                            V[:, 512 * t + 128 * g:512 * t + 128 * (g + 1)],
                            TRI,
                            start=True, stop=(t == TC - 1),
                            skip_group_check=True,
                        )
                        if t < TC - 1:
                            nc.tensor.matmul(
                                avt[:, 128 * tl:128 * (tl + 1)],
                                Bsb[:, 128 * g:128 * (g + 1)],
                                cbt[0:8, CB_TSEL + 128 * t:CB_TSEL + 128 * (t + 1)],
                                start=False, stop=True,
                                skip_group_check=True,
                            )
                    # exp numerators for the last two columns (C-part absorbs
                    # the q=S-2 contribution exactly; q=S-1 is pure exp part)
                    for s in range(2):
                        for j in range(TC):
                            nc.tensor.matmul(
                                avt[64 * s:64 * (s + 1), 510:512],
                                V[:, 512 * j + 128 * g + 64 * s:
                                   512 * j + 128 * g + 64 * (s + 1)],
                                eec[:, 16 * j + 4 * g + 2 * s:
                                    16 * j + 4 * g + 2 * s + 2],
                                start=False, stop=(j == TC - 1),
                                skip_group_check=True,
                            )
                    nc.scalar.copy(
                        av_all[:, S * g + 512:S * g + 768], avt[:, 0:256])
                    nc.vector.tensor_copy(
                        av_all[:, S * g + 768:S * g + 1024], avt[:, 256:512])
                    # per-head divide for the last two columns only
                    for s in range(2):
                        bcw = sm_bcw(g, s)
                        nc.tensor.matmul(
                            bcw,
                            cft[0:1, CF_ONES:CF_ONES + 64],
                            rr_sb[0:1, 4 * g + 2 * s:4 * g + 2 * s + 2],
                            start=True, stop=True,
                            skip_group_check=True,
                        )
                        bcw_sb = bcw_all[0:64, 2 * (2 * g + s):2 * (2 * g + s) + 2]
                        nc.vector.tensor_copy(bcw_sb, bcw)
                        nc.vector.tensor_tensor(
                            out=av_all[64 * s:64 * (s + 1),
                                       S * g + S - 2:S * g + S],
                            in0=avt[64 * s:64 * (s + 1), 510:512],
                            in1=bcw_sb,
                            op=Alu.mult,
                        )

                # output projection: per-partition 1/(C*n) scale at copy-out
                def emit_pj(t):
                    osb = outp.tile([128, E], bf16, tag="o", name=f"o_{t}")
                    rct = rcol[:, t:t + 1]
                    for c in range(2):
                        ps = mmp.tile([128, 512], f32, tag="mm", name=f"pj_{t}_{c}")
                        for g in range(NG):
                            nc.tensor.matmul(
                                ps[:],
                                av_all[:, S * g + 128 * t:S * g + 128 * (t + 1)],
                                wpt[:, E * g + 512 * c:E * g + 512 * (c + 1)],
                                start=(g == 0),
                                stop=(g == NG - 1),
                            )
                        dst = osb[:, 512 * c:512 * (c + 1)]
                        if (2 * t + c) % 2 == 0:
                            nc.scalar.activation(dst, ps[:], Act.Identity, scale=rct)
                        else:
                            nc.vector.tensor_scalar_mul(dst, ps[:], rct)
                        if not bp_zero:
                            nc.vector.tensor_tensor(
                                out=dst, in0=dst,
                                in1=bpb[:, 512 * c:512 * (c + 1)], op=Alu.add,
                            )
                        nc.sync.dma_start(
                            out_d[128 * t:128 * (t + 1), 512 * c:512 * (c + 1)].bitcast(bf16),
                            dst,
                        )

                for t in range(TC):
                    emit_pj(t)

    nc.compile()
    return nc


_last_nc = None  # program used by the most recent kernel() call (for timing)


def kernel(hidden_states, w_qkv, b_qkv, w_proj, b_proj, attn_mask):
    global _last_nc
    hidden_states = np.ascontiguousarray(np.asarray(hidden_states, dtype=np.float32))
    w_qkv = np.ascontiguousarray(np.asarray(w_qkv, dtype=np.float32))
    b_qkv = np.ascontiguousarray(np.asarray(b_qkv, dtype=np.float32))
    w_proj = np.ascontiguousarray(np.asarray(w_proj, dtype=np.float32))
    b_proj = np.ascontiguousarray(np.asarray(b_proj, dtype=np.float32))
    attn_mask = np.ascontiguousarray(np.asarray(attn_mask, dtype=np.float32))

    bq_zero = not np.any(b_qkv[0:E])
    bk_zero = not np.any(b_qkv[E:2 * E])
    bv_zero = not np.any(b_qkv[2 * E:3 * E])
    bp_zero = not np.any(b_proj)

    mask_bool = attn_mask != 0.0
    is_causal = bool(
        np.array_equal(mask_bool, np.tril(np.ones((S, S), dtype=bool)))
    ) and bk_zero

    if is_causal:
        return _kernel_causal(
            hidden_states, w_qkv, b_qkv, w_proj, b_proj,
            bq_zero=bq_zero, bv_zero=bv_zero, bp_zero=bp_zero,
        )
    return _kernel_general(
        hidden_states, w_qkv, b_qkv, w_proj, b_proj, attn_mask
    )


def _bf16_u16(a):
    return np.ascontiguousarray(
        np.asarray(a, dtype=np.float32).astype(ml_dtypes.bfloat16).view(np.uint16)
    )


def _kernel_causal(hidden_states, w_qkv, b_qkv, w_proj, b_proj,
                   bq_zero, bv_zero, bp_zero):
    global _last_nc
    key = ("causal", bq_zero, bv_zero, bp_zero)
    if key in _program_cache:
        nc = _program_cache[key]
    else:
        nc = build_program_causal(bq_zero=bq_zero, bv_zero=bv_zero, bp_zero=bp_zero)
        _program_cache[key] = nc
    _last_nc = nc

    in_maps = []
    for c in range(N_CORES):
        b, hh = c // 2, c % 2
        cols = slice(512 * hh, 512 * (hh + 1))
        cb, cf, bqs, rcol = host_const_blobs(b_qkv, b_proj, hh)
        im = {
            "hT16": _bf16_u16(hidden_states[b].T),
            "wv16": _bf16_u16(w_qkv[:, 2 * E + 512 * hh:2 * E + 512 * (hh + 1)]),
            "wq16": _bf16_u16(w_qkv[:, cols]),
            "wkT16": _bf16_u16(w_qkv[:, E + 512 * hh:E + 512 * (hh + 1)].T),
            "wp16": _bf16_u16(w_proj[cols, :]),
            "cb16": cb,
            "cf32": cf,
            "rcol32": rcol,
        }
        if not bq_zero:
            im["bqs32"] = bqs
        in_maps.append(im)

    res = run_bass_kernel_spmd(nc, in_maps, core_ids=list(range(N_CORES)))
    out = np.empty((B, S, E), dtype=np.float32)
    for b in range(B):
        o0 = np.asarray(res.results[2 * b]["out"]).view(ml_dtypes.bfloat16).astype(np.float32)
        o1 = np.asarray(res.results[2 * b + 1]["out"]).view(ml_dtypes.bfloat16).astype(np.float32)
        out[b] = o0 + o1
    return out


def _kernel_general(hidden_states, w_qkv, b_qkv, w_proj, b_proj, attn_mask):
    global _last_nc
    maskT_u8 = np.ascontiguousarray((attn_mask.T != 0.0).astype(np.uint8))
    zeros_bp = np.zeros_like(b_proj)
    in_maps = []
    for c in range(N_CORES):
        b, hh = c // 2, c % 2
        cols = slice(512 * hh, 512 * (hh + 1))
        w_half = np.ascontiguousarray(
            np.concatenate(
                [w_qkv[:, cols], w_qkv[:, E + 512 * hh:E + 512 * (hh + 1)],
                 w_qkv[:, 2 * E + 512 * hh:2 * E + 512 * (hh + 1)]],
                axis=1,
            )
        )
        b_half = np.ascontiguousarray(
            np.concatenate(
                [b_qkv[cols], b_qkv[E + 512 * hh:E + 512 * (hh + 1)],
                 b_qkv[2 * E + 512 * hh:2 * E + 512 * (hh + 1)]]
            )
        )
        in_maps.append(
            {
                "hT": np.ascontiguousarray(hidden_states[b].T),
                "w_kT_half": np.ascontiguousarray(
                    w_qkv[:, E + 512 * hh:E + 512 * (hh + 1)].T
                ),
                "maskT": maskT_u8,
                "w_qkv_half": w_half,
                "w_proj_half": np.ascontiguousarray(w_proj[cols, :]),
                "b_qkv_half": b_half,
                "b_proj_in": b_proj if hh == 0 else zeros_bp,
            }
        )

    bk_zero = not np.any(b_qkv[E:2 * E])
    bv_zero = not np.any(b_qkv[2 * E:3 * E])
    qc_modes, blk = classify_mask(attn_mask, bk_zero=bk_zero)
    nc = build_program(qc_modes, blk, bv_zero=bv_zero)
    _last_nc = nc
    res = run_bass_kernel_spmd(nc, in_maps, core_ids=list(range(N_CORES)))

    out = np.empty((B, S, E), dtype=np.float32)
    for b in range(B):
        out[b] = res.results[2 * b]["out"] + res.results[2 * b + 1]["out"]
    return out


if __name__ == "__main__":
    rng = np.random.default_rng(0)
    inputs = {
        "hidden_states": rng.standard_normal((B, S, E)).astype(np.float32),
        "w_qkv": (rng.standard_normal((E, 3 * E)) * 0.02).astype(np.float32),
        "b_qkv": np.zeros(3 * E, np.float32),
        "w_proj": (rng.standard_normal((E, E)) * 0.02).astype(np.float32),
        "b_proj": np.zeros(E, np.float32),
        "attn_mask": np.tril(np.ones((S, S), np.float32)),
    }
    out = kernel(**inputs)
    print("kernel ran, out shape", out.shape, "finite:", np.isfinite(out).all())


# revision 39
# speedup vs baseline: 1.0335x; 1.0335x over previous
"""Trainium2 Bass kernel for nn_Attention_54589034332712.

Sharding: 8 cores = 4 batches x 2 head-halves (tensor parallel over heads,
per the sharding hint).  Core c handles batch c//2 and heads
[8*(c%2), 8*(c%2)+8) for all 1024 queries.  Each core computes a partial
output projection over its 8 heads; the halves are summed at gather time
(device collectives fail to load in this environment, so the all-reduce of
the hint happens host-side as part of unsharding).

Mask specialization (exact, derived from the actual mask values at build
time, so any 0/1 mask is handled correctly):
  The reference computes w*mask - finfo.min*(1-mask): masked entries get a
  huge positive bias, so for any query row with >=1 masked entry softmax
  underflows the unmasked weights to exactly 0 and distributes uniformly
  over masked entries.  We compute P_num = exp(scores) + BT where
  BT = C*(1-maskT), C = 2^115.  For q-chunks where ALL rows have >=1 masked
  entry, P_num = BT alone is exact (unmasked weights are exactly 0 in the
  reference), so scores/exp are skipped and the AV matmul consumes BT
  directly.  Blocks with no masked entries skip the BT add.  Denominators
  come free from a ones column appended to V; division uses fp32 reciprocal
  + a rank-1 f32r broadcast matmul.  All matmuls in float32r.
"""

import sys

sys.path.insert(0, "/opt/trn_rl_repo")

import os

import numpy as np
import ml_dtypes

import concourse.bacc as bacc
import concourse.bass as bass
import concourse.mybir as mybir
import concourse.tile as tile
from concourse.bass_utils import run_bass_kernel_spmd

f32 = mybir.dt.float32
f32r = mybir.dt.float32r
bf16 = mybir.dt.bfloat16
u16 = mybir.dt.uint16
u32 = mybir.dt.uint32
Act = mybir.ActivationFunctionType
Alu = mybir.AluOpType

B, S, E, H = 4, 1024, 1024, 16
D = E // H  # 64
HH = H // 2  # heads per core (8)
NG = HH // 2  # local head groups of 2 (4)
EC = E // 128  # contraction chunks (8)
KC = S // 128  # k chunks (8)
TC = S // 128  # position chunks (causal path)
QC = S // 512  # q chunks (2)
MASK_C = float(2.0**115)
N_CORES = 8
ONE_F32_BITS = 1065353216

SC_BUFS = int(os.environ.get("KSC_BUFS", "1"))
EP_BUFS = int(os.environ.get("KEP_BUFS", "6"))
MM_BUFS = int(os.environ.get("KMM_BUFS", "2"))

_program_cache = {}


def classify_mask(attn_mask, bk_zero=True):
    """Per q-chunk execution mode + per-block mask info, uniform across cores.

    Modes per 512-row q-chunk:
      ("degen", None): every row has >=1 masked entry -> P_num = BT exactly
        (reference softmax underflows unmasked weights to exactly 0).
      ("corr", (r0, r1)): like degen except a small contiguous range of rows
        [r0, r1) has no masked entries; those columns get a dense-softmax
        correction accumulated into the AV psum.
      ("full", None): general path (scores+exp for every block, BT add where
        the block has masked entries).
    """
    m = np.asarray(attn_mask) != 0.0  # True = keep
    row_has_masked = ~m.all(axis=1)  # (S,)
    modes = []
    block_has_masked = []
    for qc in range(QC):
        rows = slice(512 * qc, 512 * (qc + 1))
        rhm = row_has_masked[rows]
        live = np.nonzero(~rhm)[0]
        if len(live) == 0:
            modes.append(("degen", None))
        elif bk_zero and len(live) <= 64 and live[-1] - live[0] + 1 == len(live):
            # f32r matmuls need even moving sizes and 8B-aligned starts; pad
            # the range into degenerate rows (their e^S contributions are
            # exactly absorbed by the 2^115 mask terms).
            r0 = int(live[0]) & ~1
            r1 = int(live[-1]) + 1
            w = r1 - r0
            w += w % 2
            if r0 + w > 512:
                r0 = 512 - w
            modes.append(("corr", (r0, r0 + w)))
        else:
            modes.append(("full", None))
        block_has_masked.append(
            tuple(
                bool((~m[rows, 128 * j : 128 * (j + 1)]).any()) for j in range(KC)
            )
        )
    return tuple(modes), tuple(block_has_masked)


def build_program(qc_modes, block_has_masked, bv_zero=False):
    key = (qc_modes, block_has_masked, bv_zero)
    if key in _program_cache:
        return _program_cache[key]
    nc = bacc.Bacc("TRN2", target_bir_lowering=False, debug=False, num_devices=N_CORES)

    hT_d = nc.dram_tensor("hT", [E, S], f32, kind="ExternalInput").ap()
    maskT_d = nc.dram_tensor("maskT", [S, S], mybir.dt.uint8, kind="ExternalInput").ap()
    wqkv_d = nc.dram_tensor("w_qkv_half", [E, 3 * 512], f32, kind="ExternalInput").ap()
    wp_d = nc.dram_tensor("w_proj_half", [512, E], f32, kind="ExternalInput").ap()
    wkT_d = nc.dram_tensor("w_kT_half", [512, E], f32, kind="ExternalInput").ap()
    bqkv_d = nc.dram_tensor("b_qkv_half", [3 * 512], f32, kind="ExternalInput").ap()
    bproj_d = nc.dram_tensor("b_proj_in", [E], f32, kind="ExternalInput").ap()
    out_d = nc.dram_tensor("out", [S, E], f32, kind="ExternalOutput").ap()

    # BT slots needed: for degenerate chunks every j; for live chunks only
    # blocks with masked entries.
    bt_slots = {}
    for qc in range(QC):
        for j in range(KC):
            if qc_modes[qc][0] in ("degen", "corr") or block_has_masked[qc][j]:
                bt_slots[(qc, j)] = len(bt_slots)
    n_bt = max(1, len(bt_slots))

    any_full = any(m == "full" for m, _ in qc_modes)
    any_corr = any(m == "corr" for m, _ in qc_modes)
    ep_bufs = EP_BUFS if any_full else 2
    with tile.TileContext(nc) as tc:
        with (
            tc.tile_pool(name="const", bufs=1) as constp,
            tc.tile_pool(name="qt", bufs=1) as qtp,
            tc.tile_pool(name="kt", bufs=1) as ktp,
            tc.tile_pool(name="vv", bufs=1) as vvp,
            tc.tile_pool(name="bt", bufs=1) as btp,
            tc.tile_pool(name="avall", bufs=1) as avallp,
        ):
            ones_f = constp.tile([1, 128], f32)
            nc.vector.memset(ones_f[:], 1.0)
            ones = constp.tile([1, 128], f32r)
            nc.vector.tensor_copy(ones[:], ones_f[:])
            onescol_f = constp.tile([128, 1], f32)
            nc.vector.memset(onescol_f[:], 1.0)
            ones_col = constp.tile([128, 1], f32r)
            nc.vector.tensor_copy(ones_col[:], onescol_f[:])
            cbias = constp.tile([128, 1], f32)
            nc.vector.memset(cbias[:], MASK_C)

            bqkv_sb = constp.tile([128, 8], f32)  # q,k biases as columns
            nc.sync.dma_start(
                bqkv_sb[:], bqkv_d[0:1024].rearrange("(c p) -> p c", p=128)
            )
            bq_s = constp.tile([128, 4], f32)
            nc.scalar.mul(bq_s[:], bqkv_sb[:, 0:4], 0.125)
            bk_r = constp.tile([128, 4], f32r)
            nc.vector.tensor_copy(bk_r[:], bqkv_sb[:, 4:8])

            bv0 = constp.tile([1, 512], f32r)
            nc.sync.dma_start(
                bv0[:],
                bqkv_d[1024:1536].rearrange("(c t) -> c t", c=1).bitcast(f32r),
            )
            bp0 = constp.tile([1, 512], f32r)
            bp1 = constp.tile([1, 512], f32r)
            nc.sync.dma_start(
                bp0[:], bproj_d[0:512].rearrange("(c t) -> c t", c=1).bitcast(f32r)
            )
            nc.sync.dma_start(
                bp1[:], bproj_d[512:E].rearrange("(c t) -> c t", c=1).bitcast(f32r)
            )

            QT = qtp.tile([128, NG * S], f32r)
            KT = ktp.tile([128, NG * S], f32r)
            V = vvp.tile([128, KC * 512], f32r)  # plain: chunk t, head h at 512t+64h
            BT = btp.tile([128, n_bt * 512], f32r)
            corr_w = {qc: rng[1] - rng[0] for qc, (m, rng) in enumerate(qc_modes) if m == "corr"}
            n_eec = max(1, sum(KC * HH * w for w in corr_w.values()))
            eec_all = btp.tile([128, n_eec], f32r)  # exp'd corr scores, (qc major) j x (h,w)
            av_all = avallp.tile([128, NG * S], f32r)

            wpp_cm = tc.tile_pool(name="wp", bufs=1)
            wpp = wpp_cm.__enter__()
            bpb = wpp.tile([128, E], f32, tag="bpb", name="bproj_bcast")
            wp_t = [
                wpp.tile([128, E], f32r, tag=f"wp{g}", name=f"wp_{g}")
                for g in range(NG)
            ]

            def _emit_wp_dmas():
                for g in range(NG):
                    nc.sync.dma_start(
                        wp_t[g][:], wp_d[128 * g : 128 * (g + 1), :].bitcast(f32r)
                    )

            # --- phase A: load + QKV ---
            with (
                tc.tile_pool(name="ht", bufs=1) as htp,
                tc.tile_pool(name="mstage", bufs=2) as msp,
                tc.tile_pool(name="wqk", bufs=4) as wqkp,
                tc.tile_pool(name="wvp", bufs=1) as wvp,
                tc.tile_pool(name="mm", bufs=MM_BUFS, space="PSUM") as mmps,
            ):
                hT = htp.tile([128, EC * S], f32r)

                def _emit_ht_dmas():
                    for e in range(EC):
                        nc.sync.dma_start(
                            hT[:, S * e : S * (e + 1)],
                            hT_d[128 * e : 128 * (e + 1), :].bitcast(f32r),
                        )
                wv = wvp.tile([128, EC * 512], f32r)

                def _emit_wv_dma():
                    for e in range(EC):
                        nc.sync.dma_start(
                            wv[:, 512 * e : 512 * (e + 1)],
                            wqkv_d[128 * e : 128 * (e + 1), 1024:1536].bitcast(f32r),
                        )

                def _emit_v():
                    for t in range(KC):
                        ps3 = mmps.tile([128, 512], f32, tag="mm")
                        for e in range(EC):
                            nc.tensor.matmul(
                                ps3[:],
                                hT[:, S * e + 128 * t : S * e + 128 * (t + 1)],
                                wv[:, 512 * e : 512 * (e + 1)],
                                start=(e == 0),
                                stop=(bv_zero and e == EC - 1),
                            )
                        if not bv_zero:
                            nc.tensor.matmul(
                                ps3[:], ones[0:1, 0:128], bv0[0:1, :],
                                start=False, stop=True,
                            )
                        nc.vector.tensor_copy(
                            V[:, 512 * t : 512 * (t + 1)], ps3[:]
                        )

                def _emit_qk_dmas(groups):
                    tiles = []
                    for g in groups:
                        wq = wqkp.tile([128, EC * 128], f32r, tag="wq", name=f"wq_{g}")
                        nc.sync.dma_start(
                            wq[:].rearrange("p (c d) -> p c d", d=128),
                            wqkv_d[:, 128 * g : 128 * (g + 1)]
                            .bitcast(f32r)
                            .rearrange("(c p) d -> p c d", p=128),
                        )
                        wk = wqkp.tile([128, EC * 128], f32r, tag="wk", name=f"wk_{g}")
                        if True:
                            nc.sync.dma_start(
                                wk[:].rearrange("p (c d) -> p c d", d=128),
                                wqkv_d[:, 512 + 128 * g : 512 + 128 * (g + 1)]
                                .bitcast(f32r)
                                .rearrange("(c p) d -> p c d", p=128),
                            )
                        tiles.append((wq, wk))
                    return tiles

                def _emit_wkT_dmas():
                    tiles = []
                    for g in range(NG):
                        wkt = wqkp.tile([128, E], f32r, tag="wkt", name=f"wkt_{g}")
                        nc.sync.dma_start(
                            wkt[:], wkT_d[128 * g : 128 * (g + 1), :].bitcast(f32r)
                        )
                        tiles.append(wkt)
                    return tiles


                def _emit_qk():
                    for g in range(NG):
                        wq, wk = _qk_tiles[g]
                        for t in range(QC):
                            mode_t, rng_t = qc_modes[t]
                            if mode_t == "full":
                                ps = mmps.tile([128, 512], f32, tag="mm")
                                for e in range(EC):
                                    nc.tensor.matmul(
                                        ps[:],
                                        wq[:, 128 * e : 128 * (e + 1)],
                                        hT[:, S * e + 512 * t : S * e + 512 * (t + 1)],
                                        start=(e == 0),
                                        stop=(e == EC - 1),
                                    )
                                nc.scalar.activation(
                                    QT[:, S * g + 512 * t : S * g + 512 * (t + 1)],
                                    ps[:],
                                    Act.Identity,
                                    bias=bq_s[:, g : g + 1],
                                    scale=0.125,
                                )
                            elif mode_t == "corr":
                                # only the live correction columns are consumed
                                r0, r1 = rng_t
                                w = r1 - r0
                                psl = mmps.tile(
                                    [128, w], f32, tag="mml", name=f"psl_{g}_{t}"
                                )
                                for e in range(EC):
                                    nc.tensor.matmul(
                                        psl[:],
                                        wq[:, 128 * e : 128 * (e + 1)],
                                        hT[
                                            :,
                                            S * e + 512 * t + r0 : S * e + 512 * t + r1,
                                        ],
                                        start=(e == 0),
                                        stop=(e == EC - 1),
                                    )
                                nc.scalar.activation(
                                    QT[
                                        :,
                                        S * g + 512 * t + r0 : S * g + 512 * t + r1,
                                    ],
                                    psl[:],
                                    Act.Identity,
                                    bias=bq_s[:, g : g + 1],
                                    scale=0.125,
                                )
                            if True:
                                ps2 = mmps.tile([128, 512], f32, tag="mm")
                                for e in range(EC):
                                    nc.tensor.matmul(
                                        ps2[:],
                                        wk[:, 128 * e : 128 * (e + 1)],
                                        hT[:, S * e + 512 * t : S * e + 512 * (t + 1)],
                                        start=(e == 0),
                                        stop=(e == EC - 1),
                                    )
                                nc.scalar.activation(
                                    KT[:, S * g + 512 * t : S * g + 512 * (t + 1)],
                                    ps2[:],
                                    Act.Identity,
                                    bias=bqkv_sb[:, 4 + g : 5 + g],
                                    scale=1.0,
                                )

                # priority order: hT (everything), wv + mask (the AV wave
                # needs only V and BT), then the QK weights (corrections only)
                def _emit_corr_scores(wkt_tiles):
                    ofs = 0
                    for qc in range(QC):
                        mode_t, rng_t = qc_modes[qc]
                        if mode_t != "corr":
                            continue
                        r0, r1 = rng_t
                        w = r1 - r0
                        hw = HH * w
                        for g in range(NG):
                            for s in range(2):
                                hloc = 2 * g + s
                                scc = mmps.tile(
                                    [128, KC * w], f32, tag="ups", name=f"scc_{qc}_{g}_{s}"
                                )
                                for j in range(KC):
                                    nc.tensor.matmul(
                                        scc[:, j * w : (j + 1) * w],
                                        KT[
                                            64 * s : 64 * (s + 1),
                                            S * g + 128 * j : S * g + 128 * (j + 1),
                                        ],
                                        QT[
                                            64 * s : 64 * (s + 1),
                                            S * g + 512 * qc + r0 : S * g + 512 * qc + r1,
                                        ],
                                        start=True,
                                        stop=True,
                                        skip_group_check=True,
                                    )
                                eout = (
                                    eec_all[:, ofs : ofs + KC * hw]
                                    .rearrange("p (j hh) -> p j hh", hh=hw)
                                    [:, :, w * hloc : w * (hloc + 1)]
                                )
                                nc.scalar.activation(
                                    eout,
                                    scc[:].rearrange("p (j wi) -> p j wi", wi=w),
                                    Act.Exp,
                                )
                        ofs += KC * hw

                def _emit_mask():
                    for (qc, j), slot in bt_slots.items():
                        mst = msp.tile([128, 512], mybir.dt.uint8, tag="mst", name=f"mst_{qc}_{j}")
                        nc.sync.dma_start(
                            mst[:],
                            maskT_d[128 * j : 128 * (j + 1), 512 * qc : 512 * (qc + 1)],
                        )
                        nc.scalar.activation(
                            BT[:, 512 * slot : 512 * (slot + 1)],
                            mst[:],
                            Act.Identity,
                            bias=cbias[:],
                            scale=-MASK_C,
                        )

                if any_full:
                    _qk_tiles = _emit_qk_dmas([0])
                    _emit_ht_dmas()
                    _qk_tiles += _emit_qk_dmas([1, 2, 3])
                    _emit_wv_dma()
                    _emit_mask()
                    _emit_qk()
                    _emit_v()
                    if any_corr:
                        _emit_corr_scores(None)
                    _emit_wp_dmas()
                else:
                    _emit_ht_dmas()
                    _emit_wv_dma()
                    _emit_mask()
                    _emit_v()
                    _qk_tiles = _emit_qk_dmas([0, 1, 2, 3])
                    _emit_qk()
                    if any_corr:
                        _emit_corr_scores(None)
                    _emit_wp_dmas()
            # --- phase B: attention (+ projection, same scope for overlap) ---
            with (
                tc.tile_pool(name="outp", bufs=4) as outp,
                tc.tile_pool(name="mm2", bufs=2, space="PSUM") as mmps2,
                tc.tile_pool(name="sc", bufs=SC_BUFS, space="PSUM") as scps,
                tc.tile_pool(name="avps", bufs=int(os.environ.get("KAV_BUFS","2")), space="PSUM") as avps,
                tc.tile_pool(name="bc", bufs=1, space="PSUM") as bcps,
                tc.tile_pool(name="ee", bufs=ep_bufs) as eep,
                tc.tile_pool(name="pp", bufs=ep_bufs) as ppp,
                tc.tile_pool(name="avtmp", bufs=2) as avtp,
                tc.tile_pool(name="rc", bufs=1) as rcp,
            ):
                recips = rcp.tile([1, HH * QC * 512], f32r)
                btden_sb = rcp.tile([1, QC * 512], f32r)
                btdraw_sb = rcp.tile([1, QC * 512], f32)
                bcast_sb = rcp.tile([128, QC * 512], f32)
                for c in range(2):
                    bq_ps = bcps.tile([128, 512], f32, tag="bc", name=f"bpb_{c}")
                    nc.tensor.matmul(
                        bq_ps[:],
                        ones[0:1, 0:128],
                        (bp0 if c == 0 else bp1)[0:1, :],
                        start=True,
                        stop=True,
                    )
                    nc.scalar.copy(bpb[:, 512 * c : 512 * (c + 1)], bq_ps[:])
                # shared denominators for BT-direct chunks: Sum_k BT[k, q]
                for qc in range(QC):
                    mode, rng = qc_modes[qc]
                    if mode == "full":
                        continue
                    btd = bcps.tile([1, 512], f32, tag="bc", name=f"btd_{qc}")
                    for j in range(KC):
                        nc.tensor.matmul(
                            btd[:],
                            ones_col[:],
                            BT[:, 512 * bt_slots[(qc, j)] : 512 * (bt_slots[(qc, j)] + 1)],
                            start=(j == 0),
                            stop=(j == KC - 1),
                        )
                    nc.scalar.copy(btdraw_sb[0:1, 512 * qc : 512 * (qc + 1)], btd[:])
                    with nc.allow_low_precision(reason="f32r recip for bcast"):
                        nc.vector.reciprocal(
                            btden_sb[0:1, 512 * qc : 512 * (qc + 1)], btd[:]
                        )
                    bcq = bcps.tile([128, 512], f32, tag="bc", name=f"bcq_{qc}")
                    nc.tensor.matmul(
                        bcq[:],
                        ones[0:1, 0:128],
                        btden_sb[0:1, 512 * qc : 512 * (qc + 1)],
                        start=True,
                        stop=True,
                    )
                    nc.scalar.copy(bcast_sb[:, 512 * qc : 512 * (qc + 1)], bcq[:])

                for g in range(NG):
                    for qc in range(QC):
                        mode, rng = qc_modes[qc]
                        if mode in ("degen", "corr"):
                            av2 = avps.tile([128, 512], f32, tag="av", name=f"av2_{g}_{qc}")
                            for j in range(KC):
                                nc.tensor.matmul(
                                    av2[:],
                                    V[:, 512 * j + 128 * g : 512 * j + 128 * (g + 1)],
                                    BT[
                                        :,
                                        512 * bt_slots[(qc, j)] : 512
                                        * (bt_slots[(qc, j)] + 1),
                                    ],
                                    start=(j == 0),
                                    stop=(j == KC - 1),
                                )
                            cds_s = []
                            cps_s = []
                            if mode == "corr":
                                r0, r1 = rng
                                w = r1 - r0
                                hw = HH * w
                                ofs = 0
                                for q2 in range(qc):
                                    if qc_modes[q2][0] == "corr":
                                        ofs += KC * HH * (qc_modes[q2][1][1] - qc_modes[q2][1][0])
                                for s in range(2):
                                    hloc = 2 * g + s
                                    cps = avps.tile(
                                        [64, w], f32, tag="cps", name=f"cps{g}{qc}{s}"
                                    )
                                    for j in range(KC):
                                        nc.tensor.matmul(
                                            cps[:],
                                            V[
                                                :,
                                                512 * j + 128 * g + 64 * s : 512 * j
                                                + 128 * g
                                                + 64 * (s + 1),
                                            ],
                                            eec_all[
                                                :,
                                                ofs + hw * j + w * hloc : ofs
                                                + hw * j
                                                + w * (hloc + 1),
                                            ],
                                            start=(j == 0),
                                            stop=(j == KC - 1),
                                        )
                                    cps_s.append(cps)
                                    # per-head denominator delta for corr cols
                                    cd = bcps.tile(
                                        [1, w], f32, tag="bc", name=f"cd{g}{qc}{s}"
                                    )
                                    for j in range(KC):
                                        nc.tensor.matmul(
                                            cd[:],
                                            ones_col[:],
                                            eec_all[
                                                :,
                                                ofs + hw * j + w * hloc : ofs
                                                + hw * j
                                                + w * (hloc + 1),
                                            ],
                                            start=(j == 0),
                                            stop=(j == KC - 1),
                                            skip_group_check=True,
                                        )
                                    cds = avtp.tile(
                                        [1, w], f32, tag=f"cds{s}", name=f"cds{g}{qc}{s}"
                                    )
                                    nc.vector.tensor_copy(cds[:], cd[:])
                                    cds_s.append(cds)
                            # divide (writes garbage into corr cols; fixed below)
                            nc.vector.tensor_tensor(
                                out=av_all[:, S * g + 512 * qc : S * g + 512 * (qc + 1)],
                                in0=av2[:],
                                in1=bcast_sb[:, 512 * qc : 512 * (qc + 1)],
                                op=Alu.mult,
                            )
                            if mode == "corr":
                                r0, r1 = rng
                                w = r1 - r0
                                for s in range(2):
                                    # combined numerator: BT part (av2) + e^S
                                    # part (cps); fp32 add swallows exactly the
                                    # right one on both degenerate-padded and
                                    # live rows.
                                    cps_sb = avtp.tile(
                                        [64, w], f32, tag="cpssb", name=f"cb{g}{qc}{s}"
                                    )
                                    nc.scalar.copy(cps_sb[:], cps_s[s][:])
                                    val_sb = avtp.tile(
                                        [64, w], f32, tag="valsb", name=f"vl{g}{qc}{s}"
                                    )
                                    nc.vector.tensor_tensor(
                                        out=val_sb[:],
                                        in0=av2[64 * s : 64 * (s + 1), r0:r1],
                                        in1=cps_sb[:],
                                        op=Alu.add,
                                    )
                                    dcomb = avtp.tile(
                                        [1, w], f32, tag="dcomb", name=f"dc{g}{qc}{s}"
                                    )
                                    nc.vector.tensor_tensor(
                                        out=dcomb[:],
                                        in0=cds_s[s][:],
                                        in1=btdraw_sb[
                                            0:1,
                                            512 * qc + r0 : 512 * qc + r1,
                                        ],
                                        op=Alu.add,
                                    )
                                    rcw = avtp.tile(
                                        [1, w], f32r, tag="rcw", name=f"rcw{g}{qc}{s}"
                                    )
                                    with nc.allow_low_precision(reason="recip"):
                                        nc.vector.reciprocal(rcw[:], dcomb[:])
                                    bcw = bcps.tile(
                                        [64, w], f32, tag="bc", name=f"bcw{g}{qc}{s}"
                                    )
                                    nc.tensor.matmul(
                                        bcw[:], ones[0:1, 0:64], rcw[:],
                                        start=True, stop=True,
                                    )
                                    bcw_sb = avtp.tile(
                                        [64, w], f32, tag="bcwsb", name=f"bw{g}{qc}{s}"
                                    )
                                    nc.scalar.copy(bcw_sb[:], bcw[:])
                                    nc.vector.tensor_tensor(
                                        out=av_all[
                                            64 * s : 64 * (s + 1),
                                            S * g + 512 * qc + r0 : S * g
                                            + 512 * qc
                                            + r1,
                                        ],
                                        in0=val_sb[:],
                                        in1=bcw_sb[:],
                                        op=Alu.mult,
                                    )
                        else:
                            # full path: per-head scores/exp/(BT add)/AV + denom
                            # (s passes deinterleaved so one dn slot suffices)
                            av_t = [
                                avps.tile([64, 512], f32, tag="av", name=f"avf_{g}_{qc}_{s}")
                                for s in range(2)
                            ]
                            for s in range(2):
                                dn = bcps.tile(
                                    [1, 512], f32, tag="bc", name=f"dn_{g}_{qc}_{s}"
                                )
                                for j in range(KC):
                                    sc = scps.tile(
                                        [128, 512], f32, tag="sc", name=f"sc{g}{qc}{j}{s}"
                                    )
                                    nc.tensor.matmul(
                                        sc[:],
                                        KT[
                                            64 * s : 64 * (s + 1),
                                            S * g + 128 * j : S * g + 128 * (j + 1),
                                        ],
                                        QT[
                                            64 * s : 64 * (s + 1),
                                            S * g + 512 * qc : S * g + 512 * (qc + 1),
                                        ],
                                        start=True,
                                        stop=True,
                                    )
                                    ee = eep.tile(
                                        [128, 512], f32r, tag="ee", name=f"ee{g}{qc}{j}{s}"
                                    )
                                    nc.scalar.activation(ee[:], sc[:], Act.Exp)
                                    if block_has_masked[qc][j]:
                                        pp = ppp.tile(
                                            [128, 512], f32r, tag="pp",
                                            name=f"pp{g}{qc}{j}{s}",
                                        )
                                        nc.vector.tensor_tensor(
                                            out=pp[:],
                                            in0=ee[:],
                                            in1=BT[
                                                :,
                                                512 * bt_slots[(qc, j)] : 512
                                                * (bt_slots[(qc, j)] + 1),
                                            ],
                                            op=Alu.add,
                                        )
                                        rhs = pp[:]
                                    else:
                                        rhs = ee[:]
                                    nc.tensor.matmul(
                                        av_t[s][:],
                                        V[
                                            :,
                                            512 * j + 128 * g + 64 * s : 512 * j
                                            + 128 * g
                                            + 64 * (s + 1),
                                        ],
                                        rhs,
                                        start=(j == 0),
                                        stop=(j == KC - 1),
                                    )
                                    nc.tensor.matmul(
                                        dn[:],
                                        ones_col[:],
                                        rhs,
                                        start=(j == 0),
                                        stop=(j == KC - 1),
                                        skip_group_check=True,
                                    )
                                hq = (2 * g + s) * QC + qc
                                with nc.allow_low_precision(reason="recip"):
                                    nc.vector.reciprocal(
                                        recips[0:1, 512 * hq : 512 * (hq + 1)],
                                        dn[:],
                                    )
                            for s in range(2):
                                hq = (2 * g + s) * QC + qc
                                bc = bcps.tile(
                                    [64, 512], f32, tag="bc", name=f"bcf{g}{qc}{s}"
                                )
                                nc.tensor.matmul(
                                    bc[:],
                                    ones[0:1, 0:64],
                                    recips[0:1, 512 * hq : 512 * (hq + 1)],
                                    start=True,
                                    stop=True,
                                )
                                avt = avtp.tile(
                                    [64, 512], f32, tag="avt", name=f"avtf{g}{qc}{s}"
                                )
                                nc.vector.tensor_copy(avt[:], av_t[s][:])
                                nc.vector.tensor_tensor(
                                    out=av_all[
                                        64 * s : 64 * (s + 1),
                                        S * g + 512 * qc : S * g + 512 * (qc + 1),
                                    ],
                                    in0=avt[:],
                                    in1=bc[:],
                                    op=Alu.mult,
                                )

                for t in range(KC):
                    out_sb = outp.tile([128, E], f32, tag="out", name=f"out_{t}")
                    for c in range(2):
                        if (2 * t + c) % 2 == 0:
                            ps = mmps2.tile([128, 512], f32, tag="mm2", name=f"pj_{t}_{c}")
                        else:
                            ps = avps.tile([128, 512], f32, tag="cps", name=f"pj_{t}_{c}")
                        for g in range(NG):
                            nc.tensor.matmul(
                                ps[:],
                                av_all[:, S * g + 128 * t : S * g + 128 * (t + 1)],
                                wp_t[g][:, 512 * c : 512 * (c + 1)],
                                start=(g == 0),
                                stop=(g == NG - 1),
                            )
                        nc.vector.scalar_tensor_tensor(
                            out=out_sb[:, 512 * c : 512 * (c + 1)],
                            in0=ps[:],
                            scalar=1.0,
                            in1=bpb[:, 512 * c : 512 * (c + 1)],
                            op0=Alu.mult,
                            op1=Alu.add,
                        )
                        nc.sync.dma_start(
                            out_d[128 * t : 128 * (t + 1), 512 * c : 512 * (c + 1)],
                            out_sb[:, 512 * c : 512 * (c + 1)],
                        )
            wpp_cm.__exit__(None, None, None)

    nc.compile()
    _program_cache[key] = nc
    return nc


N_WARM = int(os.environ.get("KN_WARM", "7"))

# const blob layouts
CB_TRI = 0            # [128, 128] strict-upper (k>q) valued C, bf16
CB_ESEL = 128         # [128, 64]: per t, col block [8] with col t' = (t'==t)
CB_TSEL = 192         # [8, 1024]: per t, [8,128] block, row t' = C*(t'>t)
CB_ONEC = 1216        # [128, 1] ones column
CB_ID = 1220          # [128, 128] identity (for PE transpose)
CB_W = 1348

CF_RECIPN = 0         # [1, 1024] 1/(C*(S-1-q)), 0 at q=S-1
CF_DENC = 1024        # [1, 16] per (g,s,q) denominator constant: C for q=0, 0 for q=1
CF_ONES = 1040        # [1, 128] ones (f32)
CF_BV = 1168          # [1, 512] b_v half
CF_BP = 1680          # [1, 1024] b_proj (zeroed on half 1)
CF_W = 2704


def host_const_blobs(b_qkv, b_proj, hh):
    """Per-core const blobs (bf16 blob as uint16 view, f32 row blob)."""
    cb = np.zeros((128, CB_W), dtype=ml_dtypes.bfloat16)
    k_idx = np.arange(128)[:, None]
    q_idx = np.arange(128)[None, :]
    cb[:, CB_TRI:CB_TRI + 128] = np.where(k_idx > q_idx, MASK_C, 0.0)
    for t in range(TC):
        cb[:, CB_ESEL + 8 * t + t] = 1.0
        tp = np.arange(8)[:, None]
        cb[0:8, CB_TSEL + 128 * t:CB_TSEL + 128 * (t + 1)] = np.where(
            tp > t, MASK_C, 0.0
        )
    cb[:, CB_ONEC] = 1.0
    cb[:, CB_ID:CB_ID + 128] = np.eye(128, dtype=np.float32)

    cf = np.zeros((1, CF_W), dtype=np.float32)
    n = (S - 1) - np.arange(S).astype(np.float64)
    with np.errstate(divide="ignore"):
        recipn = np.where(n > 0, 1.0 / (MASK_C * np.maximum(n, 1.0)), 0.0)
    cf[0, CF_RECIPN:CF_RECIPN + S] = recipn.astype(np.float32)
    denc = np.zeros(16, dtype=np.float32)
    denc[0::2] = MASK_C  # q==0 columns (row S-2): one masked key
    cf[0, CF_DENC:CF_DENC + 16] = denc
    cf[0, CF_ONES:CF_ONES + 128] = 1.0
    cf[0, CF_BV:CF_BV + 512] = b_qkv[2 * E + 512 * hh:2 * E + 512 * (hh + 1)]
    if hh == 0:
        cf[0, CF_BP:CF_BP + E] = b_proj
    bq = b_qkv[512 * hh:512 * (hh + 1)].astype(np.float32)
    bqs = np.ascontiguousarray(0.125 * bq.reshape(4, 128).T)
    rcol = np.where(n > 0, 1.0 / (MASK_C * np.maximum(n, 1.0)), 1.0)
    rcol[S - 2] = 1.0  # last two queries are pre-divided by the fixup
    rcol = np.ascontiguousarray(rcol.astype(np.float32).reshape(TC, 128).T)
    return (np.ascontiguousarray(cb.view(np.uint16)), np.ascontiguousarray(cf),
            bqs, rcol)


def build_program_causal(bq_zero=True, bv_zero=True, bp_zero=True):
    nc = bacc.Bacc("TRN2", target_bir_lowering=False, debug=False,
                   num_devices=N_CORES)

    hT_d = nc.dram_tensor("hT16", [E, S], u16, kind="ExternalInput").ap()
    wv_d = nc.dram_tensor("wv16", [E, 512], u16, kind="ExternalInput").ap()
    wq_d = nc.dram_tensor("wq16", [E, 512], u16, kind="ExternalInput").ap()
    wkT_d = nc.dram_tensor("wkT16", [512, E], u16, kind="ExternalInput").ap()
    wp_d = nc.dram_tensor("wp16", [512, E], u16, kind="ExternalInput").ap()
    cb_d = nc.dram_tensor("cb16", [128, CB_W], u16, kind="ExternalInput").ap()
    cf_d = nc.dram_tensor("cf32", [1, CF_W], f32, kind="ExternalInput").ap()
    rcol_d = nc.dram_tensor("rcol32", [128, TC], f32, kind="ExternalInput").ap()
    if not bq_zero:
        bqs_d = nc.dram_tensor("bqs32", [128, 4], f32, kind="ExternalInput").ap()
    out_d = nc.dram_tensor("out", [S, E], u16, kind="ExternalOutput").ap()

    need_ones = (not bv_zero) or (not bp_zero)

    with tile.TileContext(nc) as tc:
        with (
            tc.tile_pool(name="const", bufs=1) as constp,
            tc.tile_pool(name="big", bufs=1) as bigp,
            tc.tile_pool(name="outp", bufs=4) as outp,
        ):
            # ---- SBUF tiles ----
            cbt = constp.tile([128, CB_W], bf16)
            cft = constp.tile([1, CF_W], f32)
            rcol = constp.tile([128, TC], f32)
            if not bq_zero:
                bqs = constp.tile([128, 4], f32)
            ones_f = constp.tile([128, 128], f32)  # warmup operand (memset)
            qbd = constp.tile([128, 16], bf16)
            Usb = constp.tile([128, EC * 16], bf16)
            eec = constp.tile([128, TC * 16], bf16)
            Bsb = constp.tile([8, 512], bf16)
            rr_sb = constp.tile([1, 16], f32)
            dcomb_sb = constp.tile([1, 16], f32)
            bcw_all = constp.tile([64, 16], f32)
            psl_sb = constp.tile([128, 8], bf16)
            BTsb = constp.tile([128, 4 * TC], bf16)
            R = constp.tile([128, EC * TC], f32)
            Rb = constp.tile([128, EC * TC], bf16)
            if need_ones:
                onesr = constp.tile([1, 128], f32r)
            if not bv_zero:
                bv_r = constp.tile([1, 512], f32r)
            if not bp_zero:
                bp_r = constp.tile([1, E], f32r)
                bpb = constp.tile([128, E], f32)

            hT = bigp.tile([128, EC * S], bf16)
            wvt = bigp.tile([128, EC * 512], bf16)
            wqt = bigp.tile([128, EC * 512], bf16)
            wkTt = bigp.tile([128, NG * E], bf16)
            wpt = bigp.tile([128, NG * E], bf16)
            V = bigp.tile([128, TC * 512], bf16)
            av_all = bigp.tile([128, NG * S], bf16)

            TRI = cbt[:, CB_TRI:CB_TRI + 128]
            onec_bf = cbt[:, CB_ONEC:CB_ONEC + 1]

            # ---- DMA issue (SP: hT; Act: wv + consts + weights) ----
            nc.gpsimd.memset(ones_f[:], 1.0)  # warmup operand, Pool, no DMA dep
            nc.gpsimd.memset(qbd[:], 0.0)
            for e in range(EC):
                nc.scalar.dma_start(
                    wvt[:, 512 * e:512 * (e + 1)],
                    wv_d[128 * e:128 * (e + 1), :].bitcast(bf16),
                )
                nc.sync.dma_start(
                    hT[:, S * e:S * (e + 1)],
                    hT_d[128 * e:128 * (e + 1), :].bitcast(bf16),
                )
            nc.scalar.dma_start(cft[:], cf_d)
            nc.scalar.dma_start(rcol[:], rcol_d)
            if not bq_zero:
                nc.scalar.dma_start(bqs[:], bqs_d)
            nc.scalar.dma_start(
                wqt[:].rearrange("p (c d) -> p c d", d=512),
                wq_d.bitcast(bf16).rearrange("(c p) d -> p c d", p=128),
            )
            nc.scalar.dma_start(cbt[:], cb_d.bitcast(bf16))
            def emit_wp(c):
                nc.scalar.dma_start(
                    wpt[:].rearrange("p (g e) -> p g e", e=E)[:, :, 512 * c:512 * (c + 1)],
                    wp_d.bitcast(bf16).rearrange("(g p) e -> p g e", p=128)[
                        :, :, 512 * c:512 * (c + 1)
                    ],
                )

            emit_wp(0)
            nc.scalar.dma_start(
                wkTt[:].rearrange("p (g e) -> p g e", e=E),
                wkT_d.bitcast(bf16).rearrange("(g p) e -> p g e", p=128),
            )
            emit_wp(1)
            for e in range(EC):
                nc.vector.tensor_reduce(
                    R[:, TC * e:TC * (e + 1)],
                    hT[:, S * e:S * (e + 1)].rearrange("p (t k) -> p t k", k=128),
                    axis=mybir.AxisListType.X,
                    op=Alu.add,
                )
            nc.vector.tensor_copy(Rb[:], R[:])
            if need_ones:
                nc.vector.tensor_copy(onesr[:], cft[0:1, CF_ONES:CF_ONES + 128])
            if not bv_zero:
                nc.vector.tensor_copy(bv_r[:], cft[0:1, CF_BV:CF_BV + 512])
            if not bp_zero:
                nc.vector.tensor_copy(bp_r[:], cft[0:1, CF_BP:CF_BP + E])

            # ---- phase A: warmup + V projection (8 psum banks) ----
            with tc.tile_pool(name="vps", bufs=8, space="PSUM") as vpool:
                warm = vpool.tile([128, 512], f32, tag="vp", name="warm")
                for i in range(N_WARM):
                    nc.tensor.matmul(
                        warm[:, 0:128], ones_f[:], ones_f[:],
                        start=True, stop=True,
                    )
                vt = [
                    vpool.tile([128, 512], f32, tag="vp", name=f"v_{t}")
                    for t in range(TC)
                ]
                for e in range(EC - 2):
                    for t in range(TC):
                        nc.tensor.matmul(
                            vt[t][:],
                            hT[:, S * e + 128 * t:S * e + 128 * (t + 1)],
                            wvt[:, 512 * e:512 * (e + 1)],
                            start=(e == 0),
                            stop=False,
                        )
                # tail skew: finish each tile and launch its copy early
                for t in range(TC):
                    for e in (EC - 2, EC - 1):
                        nc.tensor.matmul(
                            vt[t][:],
                            hT[:, S * e + 128 * t:S * e + 128 * (t + 1)],
                            wvt[:, 512 * e:512 * (e + 1)],
                            start=False,
                            stop=(bv_zero and e == EC - 1),
                        )
                    if not bv_zero:
                        nc.tensor.matmul(
                            vt[t][:], onesr[:], bv_r[:], start=False, stop=True,
                        )
                    dst = V[:, 512 * t:512 * (t + 1)]
                    if t % 2 == 0:
                        nc.scalar.copy(dst, vt[t][:])
                    else:
                        nc.vector.tensor_copy(dst, vt[t][:])

                # block column sums, transposed: B^T[d, t'] = Wv^T R
                # (tiny free dim), then PE-transpose back to [t', d]
                pslt = vpool.tile([128, 512], f32, tag="vp", name="pslt")
                for gd in range(4):
                    for e in range(EC):
                        nc.tensor.matmul(
                            pslt[:, 16 + 8 * gd:24 + 8 * gd],
                            wvt[:, 512 * e + 128 * gd:512 * e + 128 * (gd + 1)],
                            Rb[:, TC * e:TC * (e + 1)],
                            start=(e == 0),
                            stop=(e == EC - 1),
                            skip_group_check=True,
                        )
                nc.scalar.copy(BTsb[:], pslt[:, 16:48])
                bsbT = vpool.tile([8, 512], bf16, tag="vp", name="bsbT")
                for gd in range(4):
                    nc.tensor.transpose(
                        bsbT[:, 128 * gd:128 * (gd + 1)],
                        BTsb[:, 8 * gd:8 * (gd + 1)],
                        cbt[:, CB_ID:CB_ID + 128],
                    )
                nc.scalar.copy(Bsb[:, 0:256], bsbT[:, 0:256])
                nc.vector.tensor_copy(Bsb[:, 256:512], bsbT[:, 256:512])
                for g in range(NG):
                    for e in range(EC):
                        nc.tensor.matmul(
                            pslt[:, 2 * g:2 * g + 2],
                            wqt[:, 512 * e + 128 * g:512 * e + 128 * (g + 1)],
                            hT[:, S * e + (S - 2):S * e + S],
                            start=(e == 0),
                            stop=(e == EC - 1),
                            skip_group_check=True,
                        )
                if bq_zero:
                    nc.scalar.activation(psl_sb[:], pslt[:, 0:8], Act.Identity, scale=0.125)
                else:
                    for g in range(NG):
                        nc.scalar.activation(
                            psl_sb[:, 2 * g:2 * g + 2], pslt[:, 2 * g:2 * g + 2],
                            Act.Identity, scale=0.125,
                            bias=bqs[:, g:g + 1],
                        )
                for g in range(NG):
                    for s in range(2):
                        nc.gpsimd.tensor_copy(
                            qbd[64 * s:64 * (s + 1), 4 * g + 2 * s:4 * g + 2 * s + 2],
                            psl_sb[64 * s:64 * (s + 1), 2 * g:2 * g + 2],
                        )

            # ---- phase B ----
            with (
                tc.tile_pool(name="mm", bufs=3, space="PSUM") as mmp,
                tc.tile_pool(name="avp", bufs=4, space="PSUM") as avp,
                tc.tile_pool(name="sm", bufs=1, space="PSUM") as smp,
            ):
                # one psum bank shared by all small tiles, via column regions
                sm = smp.tile([128, 512], f32, tag="sm")
                sm_psl = lambda g: sm[:, 2 * g:2 * g + 2]
                sm_u = lambda e: sm[:, 32 + 16 * e:48 + 16 * e]
                sm_s = lambda j: sm[:, 160 + 16 * j:176 + 16 * j]
                sm_cd = sm[0:1, 288:304]
                sm_bcw = lambda g, s: sm[0:64, 304 + 2 * (2 * g + s):306 + 2 * (2 * g + s)]

                if not bp_zero:
                    for c in range(2):
                        bps = mmp.tile([128, 512], f32, tag="mm", name=f"bpb_{c}")
                        nc.tensor.matmul(
                            bps[:], onesr[:], bp_r[0:1, 512 * c:512 * (c + 1)],
                            start=True, stop=True,
                        )
                        nc.scalar.copy(bpb[:, 512 * c:512 * (c + 1)], bps[:])

                # qc0 attention numerators (queries 0..511), C-scaled
                # NOTE: keep each region's start..stop pair adjacent: interleaving
                # open accumulation groups across tiles miscomputes in this stack.
                for g in range(NG):
                    avt = avp.tile([128, 512], f32, tag="av", name=f"av0_{g}")
                    for tl in range(4):
                        nc.tensor.matmul(
                            avt[:, 128 * tl:128 * (tl + 1)],
                            V[:, 512 * tl + 128 * g:512 * tl + 128 * (g + 1)],
                            TRI,
                            start=True, stop=False,
                            skip_group_check=True,
                        )
                        nc.tensor.matmul(
                            avt[:, 128 * tl:128 * (tl + 1)],
                            Bsb[:, 128 * g:128 * (g + 1)],
                            cbt[0:8, CB_TSEL + 128 * tl:CB_TSEL + 128 * (tl + 1)],
                            start=False, stop=True,
                            skip_group_check=True,
                        )
                    nc.scalar.copy(
                        av_all[:, S * g:S * g + 256], avt[:, 0:256])
                    nc.vector.tensor_copy(
                        av_all[:, S * g + 256:S * g + 512], avt[:, 256:512])

                # u = Wk^T q (block-diagonal via zero-padded qbd)
                for e in range(EC):
                    ups = sm_u(e)
                    for g in range(NG):
                        nc.tensor.matmul(
                            ups[:, 4 * g:4 * (g + 1)],
                            wkTt[:, E * g + 128 * e:E * g + 128 * (e + 1)],
                            qbd[:, 4 * g:4 * (g + 1)],
                            start=True, stop=True,
                            skip_group_check=True,
                        )
                nc.scalar.copy(Usb[:], sm[:, 32:160])

                # s = H^T u; exp -> eec
                for j in range(TC):
                    sps = sm_s(j)
                    for e in range(EC):
                        nc.tensor.matmul(
                            sps,
                            hT[:, S * e + 128 * j:S * e + 128 * (j + 1)],
                            Usb[:, 16 * e:16 * (e + 1)],
                            start=(e == 0),
                            stop=(e == EC - 1),
                            skip_group_check=True,
                        )
                nc.scalar.activation(eec[:], sm[:, 160:288], Act.Exp)

                # denominators for the live row (+ absorbed degenerate col)
                cd = sm_cd
                for j in range(TC):
                    nc.tensor.matmul(
                        cd, onec_bf, eec[:, 16 * j:16 * (j + 1)],
                        start=(j == 0), stop=(j == TC - 1),
                        skip_group_check=True,
                    )
                nc.vector.tensor_tensor(
                    out=dcomb_sb[:], in0=cd,
                    in1=cft[0:1, CF_DENC:CF_DENC + 16], op=Alu.add,
                )
                with nc.allow_low_precision(reason="corr recip"):
                    nc.vector.reciprocal(rr_sb[:], dcomb_sb[:])

                # qc1 attention numerators (queries 512..1023) + correction
                for g in range(NG):
                    avt = avp.tile([128, 512], f32, tag="av", name=f"av1_{g}")
                    for tl in range(4):
                        t = 4 + tl
                        nc.tensor.matmul(
                            avt[:, 128 * tl:128 * (tl + 1)],
                            V[:, 512 * t + 128 * g:512 * t + 128 * (g + 1)],
                            TRI,
                            start=True, stop=(t == TC - 1),
                            skip_group_check=True,
                        )
                        if t < TC - 1:
                            nc.tensor.matmul(
                                avt[:, 128 * tl:128 * (tl + 1)],
                                Bsb[:, 128 * g:128 * (g + 1)],
                                cbt[0:8, CB_TSEL + 128 * t:CB_TSEL + 128 * (t + 1)],
                                start=False, stop=True,
                                skip_group_check=True,
                            )
                    # exp numerators for the last two columns (C-part absorbs
                    # the q=S-2 contribution exactly; q=S-1 is pure exp part)
                    for s in range(2):
                        for j in range(TC):
                            nc.tensor.matmul(
                                avt[64 * s:64 * (s + 1), 510:512],
                                V[:, 512 * j + 128 * g + 64 * s:
                                   512 * j + 128 * g + 64 * (s + 1)],
                                eec[:, 16 * j + 4 * g + 2 * s:
                                    16 * j + 4 * g + 2 * s + 2],
                                start=False, stop=(j == TC - 1),
                                skip_group_check=True,
                            )
                    nc.scalar.copy(
                        av_all[:, S * g + 512:S * g + 768], avt[:, 0:256])
                    nc.vector.tensor_copy(
                        av_all[:, S * g + 768:S * g + 1024], avt[:, 256:512])
                    # per-head divide for the last two columns only
                    for s in range(2):
                        bcw = sm_bcw(g, s)
                        nc.tensor.matmul(
                            bcw,
                            cft[0:1, CF_ONES:CF_ONES + 64],
                            rr_sb[0:1, 4 * g + 2 * s:4 * g + 2 * s + 2],
                            start=True, stop=True,
                            skip_group_check=True,
                        )
                        bcw_sb = bcw_all[0:64, 2 * (2 * g + s):2 * (2 * g + s) + 2]
                        nc.vector.tensor_copy(bcw_sb, bcw)
                        nc.vector.tensor_tensor(
                            out=av_all[64 * s:64 * (s + 1),
                                       S * g + S - 2:S * g + S],
                            in0=avt[64 * s:64 * (s + 1), 510:512],
                            in1=bcw_sb,
                            op=Alu.mult,
                        )

                # output projection: per-partition 1/(C*n) scale at copy-out
                def emit_pj(t):
                    osb = outp.tile([128, E], bf16, tag="o", name=f"o_{t}")
                    rct = rcol[:, t:t + 1]
                    for c in range(2):
                        ps = mmp.tile([128, 512], f32, tag="mm", name=f"pj_{t}_{c}")
                        for g in range(NG):
                            nc.tensor.matmul(
                                ps[:],
                                av_all[:, S * g + 128 * t:S * g + 128 * (t + 1)],
                                wpt[:, E * g + 512 * c:E * g + 512 * (c + 1)],
                                start=(g == 0),
                                stop=(g == NG - 1),
                            )
                        dst = osb[:, 512 * c:512 * (c + 1)]
                        if (2 * t + c) % 2 == 0:
                            nc.scalar.activation(dst, ps[:], Act.Identity, scale=rct)
                        else:
                            nc.vector.tensor_scalar_mul(dst, ps[:], rct)
                        if not bp_zero:
                            nc.vector.tensor_tensor(
                                out=dst, in0=dst,
                                in1=bpb[:, 512 * c:512 * (c + 1)], op=Alu.add,
                            )
                        nc.sync.dma_start(
                            out_d[128 * t:128 * (t + 1), 512 * c:512 * (c + 1)].bitcast(bf16),
                            dst,
                        )

                for t in range(TC):
                    emit_pj(t)

    nc.compile()
    return nc


_last_nc = None  # program used by the most recent kernel() call (for timing)


def kernel(hidden_states, w_qkv, b_qkv, w_proj, b_proj, attn_mask):
    global _last_nc
    hidden_states = np.ascontiguousarray(np.asarray(hidden_states, dtype=np.float32))
    w_qkv = np.ascontiguousarray(np.asarray(w_qkv, dtype=np.float32))
    b_qkv = np.ascontiguousarray(np.asarray(b_qkv, dtype=np.float32))
    w_proj = np.ascontiguousarray(np.asarray(w_proj, dtype=np.float32))
    b_proj = np.ascontiguousarray(np.asarray(b_proj, dtype=np.float32))
    attn_mask = np.ascontiguousarray(np.asarray(attn_mask, dtype=np.float32))

    bq_zero = not np.any(b_qkv[0:E])
    bk_zero = not np.any(b_qkv[E:2 * E])
    bv_zero = not np.any(b_qkv[2 * E:3 * E])
    bp_zero = not np.any(b_proj)

    mask_bool = attn_mask != 0.0
    is_causal = bool(
        np.array_equal(mask_bool, np.tril(np.ones((S, S), dtype=bool)))
    ) and bk_zero

    if is_causal:
        return _kernel_causal(
            hidden_states, w_qkv, b_qkv, w_proj, b_proj,
            bq_zero=bq_zero, bv_zero=bv_zero, bp_zero=bp_zero,
        )
    return _kernel_general(
        hidden_states, w_qkv, b_qkv, w_proj, b_proj, attn_mask
    )


def _bf16_u16(a):
    return np.ascontiguousarray(
        np.asarray(a, dtype=np.float32).astype(ml_dtypes.bfloat16).view(np.uint16)
    )


def _kernel_causal(hidden_states, w_qkv, b_qkv, w_proj, b_proj,
                   bq_zero, bv_zero, bp_zero):
    global _last_nc
    key = ("causal", bq_zero, bv_zero, bp_zero)
    if key in _program_cache:
        nc = _program_cache[key]
    else:
        nc = build_program_causal(bq_zero=bq_zero, bv_zero=bv_zero, bp_zero=bp_zero)
        _program_cache[key] = nc
    _last_nc = nc

    in_maps = []
    for c in range(N_CORES):
        b, hh = c // 2, c % 2
        cols = slice(512 * hh, 512 * (hh + 1))
        cb, cf, bqs, rcol = host_const_blobs(b_qkv, b_proj, hh)
        im = {
            "hT16": _bf16_u16(hidden_states[b].T),
            "wv16": _bf16_u16(w_qkv[:, 2 * E + 512 * hh:2 * E + 512 * (hh + 1)]),
            "wq16": _bf16_u16(w_qkv[:, cols]),
            "wkT16": _bf16_u16(w_qkv[:, E + 512 * hh:E + 512 * (hh + 1)].T),
            "wp16": _bf16_u16(w_proj[cols, :]),
            "cb16": cb,
            "cf32": cf,
            "rcol32": rcol,
        }
        if not bq_zero:
            im["bqs32"] = bqs
        in_maps.append(im)

    res = run_bass_kernel_spmd(nc, in_maps, core_ids=list(range(N_CORES)))
    out = np.empty((B, S, E), dtype=np.float32)
    for b in range(B):
        o0 = np.asarray(res.results[2 * b]["out"]).view(ml_dtypes.bfloat16).astype(np.float32)
        o1 = np.asarray(res.results[2 * b + 1]["out"]).view(ml_dtypes.bfloat16).astype(np.float32)
        out[b] = o0 + o1
    return out


def _kernel_general(hidden_states, w_qkv, b_qkv, w_proj, b_proj, attn_mask):
    global _last_nc
    maskT_u8 = np.ascontiguousarray((attn_mask.T != 0.0).astype(np.uint8))
    zeros_bp = np.zeros_like(b_proj)
    in_maps = []
    for c in range(N_CORES):
        b, hh = c // 2, c % 2
        cols = slice(512 * hh, 512 * (hh + 1))
        w_half = np.ascontiguousarray(
            np.concatenate(
                [w_qkv[:, cols], w_qkv[:, E + 512 * hh:E + 512 * (hh + 1)],
                 w_qkv[:, 2 * E + 512 * hh:2 * E + 512 * (hh + 1)]],
                axis=1,
            )
        )
        b_half = np.ascontiguousarray(
            np.concatenate(
                [b_qkv[cols], b_qkv[E + 512 * hh:E + 512 * (hh + 1)],
                 b_qkv[2 * E + 512 * hh:2 * E + 512 * (hh + 1)]]
            )
        )
        in_maps.append(
            {
                "hT": np.ascontiguousarray(hidden_states[b].T),
                "w_kT_half": np.ascontiguousarray(
                    w_qkv[:, E + 512 * hh:E + 512 * (hh + 1)].T
                ),
                "maskT": maskT_u8,
                "w_qkv_half": w_half,
                "w_proj_half": np.ascontiguousarray(w_proj[cols, :]),
                "b_qkv_half": b_half,
                "b_proj_in": b_proj if hh == 0 else zeros_bp,
            }
        )

    bk_zero = not np.any(b_qkv[E:2 * E])
    bv_zero = not np.any(b_qkv[2 * E:3 * E])
    qc_modes, blk = classify_mask(attn_mask, bk_zero=bk_zero)
    nc = build_program(qc_modes, blk, bv_zero=bv_zero)
    _last_nc = nc
    res = run_bass_kernel_spmd(nc, in_maps, core_ids=list(range(N_CORES)))

    out = np.empty((B, S, E), dtype=np.float32)
    for b in range(B):
        out[b] = res.results[2 * b]["out"] + res.results[2 * b + 1]["out"]
    return out


if __name__ == "__main__":
    rng = np.random.default_rng(0)
    inputs = {
        "hidden_states": rng.standard_normal((B, S, E)).astype(np.float32),
        "w_qkv": (rng.standard_normal((E, 3 * E)) * 0.02).astype(np.float32),
        "b_qkv": np.zeros(3 * E, np.float32),
        "w_proj": (rng.standard_normal((E, E)) * 0.02).astype(np.float32),
        "b_proj": np.zeros(E, np.float32),
        "attn_mask": np.tril(np.ones((S, S), np.float32)),
    }
    out = kernel(**inputs)
    print("kernel ran, out shape", out.shape, "finite:", np.isfinite(out).all())
